# revision 13
# baseline (speedup 1.0000x reference)
"""Trainium2 Bass kernel for nn_NeuralNet_62045097558546 (topk_masking).

Network (fp32): 4-layer MLP with SOFT top-k (Sinkhorn) masking after the
first three ReLU layers.  x:[4096,1024] @ W1:[1024,500] -> mask -> @W2[500,500]
-> mask -> @W3[500,500] -> mask -> @W4[500,10].

Math: the reference's 50 Sinkhorn iterations over anchors {0,1} reduce to a
per-row scalar fixed point: solve sum_j sigmoid(c1*a_j + B) = k for the
per-row ACT bias B, where c1 = -20/Cmax and Cmax = max(M^2, 1) with M the
max activation over the FULL batch (one 8-core AllGather of a scalar per
layer).  mask = (k/s0) * sigmoid(c1*a + B).

Schedule (the point of this version): the first collective cannot complete
before ~75us (ncfw entry barrier + first-op setup), and each later AllGather
takes ~6us.  We hide Newton iterations inside those windows by PRESOLVING
with a predicted Cmax_pre = (1.13*local_max)^2 (engines are otherwise idle
while the gather is in flight), then warm-start the global solve with
B *= c1_glob/c1_pre and run 2 guarded Newton update rounds + a final fp32
sigmoid eval that directly yields the mask.  Matmul path (x, W, masked
activations, PE transposes) runs in bf16 (2x PE stream rate, half DMA);
solver input activations stay fp32.  ReLU is done on DVE/GpSimd max-copies
so ACT keeps its sigmoid table loaded (one table load, in the DMA shadow).
Layer 4 is computed transposed (out' = W4^T-chunks x am3T, free dim 512
instead of 10); the host transposes the [10,512] result back.

Validated on CPU sim vs the 50-iteration reference: rel err ~9e-3 sim-units
(sim overestimates: the same sim scores the previous 197us kernel's schedule
at ~4e-3 while it measures 7e-4 on HW).
"""

import numpy as np
from contextlib import ExitStack

BS, D_IN, D_H, D_OUT = 4096, 1024, 500, 10
NCORES = 8
BPC = BS // NCORES            # 512 batch rows per core
NBT = BPC // 128              # 4 batch tiles of 128
KC1 = D_IN // 128             # 8 contraction chunks for layer 1
CH = 125                      # contraction chunk size for 500-dim layers
KC2 = D_H // CH               # 4 chunks
K_TOPK = 400.0
PRED2_SHARD = 1.28            # (global/shard-max)^2 predictor, layer 1
PRED2_PART = 1.6              # (global/partition-max)^2 predictor, layers 2-3
PRE_ROUNDS = [3, 2, 2]        # local-Cmax presolve rounds per layer
UPD_ROUNDS = [1, 2, 2]        # global-Cmax Newton update rounds per layer
DMIN = 2.0                    # |d| floor (negated-d convention)
CAP = 8.0                     # Newton step clamp

_CACHE = {}


def _build(masked: bool, zero_bias: bool = False):
    import concourse.bass as bass
    import concourse.bacc as bacc
    import concourse.mybir as mybir
    import concourse.tile as tile
    from concourse import masks as cmasks

    f32 = mybir.dt.float32
    bf16 = mybir.dt.bfloat16
    AX = mybir.AxisListType
    OP = mybir.AluOpType
    AF = mybir.ActivationFunctionType

    nc = bacc.Bacc("TRN2", target_bir_lowering=False, debug=False,
                   num_devices=NCORES)

    xT = nc.dram_tensor("xT", [D_IN, BPC], bf16, kind="ExternalInput")
    W1 = nc.dram_tensor("W1", [D_IN, D_H], bf16, kind="ExternalInput")
    W2 = nc.dram_tensor("W2", [D_H, D_H], bf16, kind="ExternalInput")
    W3 = nc.dram_tensor("W3", [D_H, D_H], bf16, kind="ExternalInput")
    W4 = nc.dram_tensor("W4", [D_H, D_OUT], bf16, kind="ExternalInput")
    b1 = nc.dram_tensor("b1", [1, D_H], bf16, kind="ExternalInput")
    b2 = nc.dram_tensor("b2", [1, D_H], bf16, kind="ExternalInput")
    b3 = nc.dram_tensor("b3", [1, D_H], bf16, kind="ExternalInput")
    b4 = nc.dram_tensor("b4", [1, D_OUT], bf16, kind="ExternalInput")
    # transposed output [D_OUT, BPC]; the host transposes back
    out = nc.dram_tensor("out", [D_OUT, BPC], f32, kind="ExternalOutput")

    with tile.TileContext(nc) as tc, ExitStack() as ctx:
        singles = ctx.enter_context(tc.tile_pool(name="singles", bufs=1))
        a_pool = ctx.enter_context(tc.tile_pool(name="a", bufs=NBT + 1))
        yb_pool = ctx.enter_context(tc.tile_pool(name="yb", bufs=3))
        yf_pool = ctx.enter_context(tc.tile_pool(name="yf", bufs=3))
        am_pool = ctx.enter_context(tc.tile_pool(name="am", bufs=NBT))
        amt_pool = ctx.enter_context(tc.tile_pool(name="amt", bufs=2))
        st_pool = ctx.enter_context(tc.tile_pool(name="st", bufs=24))
        sc_pool = ctx.enter_context(tc.tile_pool(name="sc", bufs=2))
        ps_mm = ctx.enter_context(tc.tile_pool(name="ps_mm", bufs=3, space="PSUM"))
        ps_tr = ctx.enter_context(tc.tile_pool(name="ps_tr", bufs=2, space="PSUM"))
        ps_sm = ctx.enter_context(tc.tile_pool(name="ps_sm", bufs=1, space="PSUM"))
        dram = ctx.enter_context(tc.tile_pool(name="dram", bufs=8, space="DRAM"))

        # ---- constants ----
        identf = singles.tile([128, 128], f32, tag="identf")
        cmasks.make_identity(nc, identf[:])
        identb = singles.tile([128, 128], bf16, tag="identb")
        nc.vector.tensor_copy(identb[:], identf[:])
        ones_col = singles.tile([1, 128], f32, tag="ones")
        nc.vector.memset(ones_col[:], 1.0)
        zero4 = singles.tile([128, 4], f32, tag="zero4")
        nc.vector.memset(zero4[:], 0.0)
        junk1 = singles.tile([128, 1], f32, tag="junk1")
        nc.vector.memset(junk1[:], 0.0)

        # ---- weight / input loads (HWDGE), split over two rings ----
        xT_sb = singles.tile([128, KC1 * BPC], bf16, tag="xT")
        xT3 = xT_sb[:].rearrange("p (c f) -> p c f", c=KC1)
        xTd = xT[:].rearrange("(c p) f -> p c f", p=128)
        W1_sb = singles.tile([128, KC1 * D_H], bf16, tag="W1")
        W13 = W1_sb[:].rearrange("p (c f) -> p c f", c=KC1)
        W1d = W1[:].rearrange("(c p) f -> p c f", p=128)
        for kk in range(KC1):
            nc.sync.dma_start(out=xT3[:, kk, :], in_=xTd[:, kk, :])
            nc.scalar.dma_start(out=W13[:, kk, :], in_=W1d[:, kk, :])

        W2_sb = singles.tile([CH, KC2 * D_H], bf16, tag="W2")
        W23 = W2_sb[:].rearrange("p (c f) -> p c f", c=KC2)
        nc.sync.dma_start(out=W23, in_=W2[:].rearrange("(c p) f -> p c f", p=CH))

        W3_sb = singles.tile([CH, KC2 * D_H], bf16, tag="W3")
        W33 = W3_sb[:].rearrange("p (c f) -> p c f", c=KC2)
        nc.scalar.dma_start(out=W33, in_=W3[:].rearrange("(c p) f -> p c f", p=CH))

        W4_sb = singles.tile([CH, KC2 * D_OUT], bf16, tag="W4")
        W43 = W4_sb[:].rearrange("p (c f) -> p c f", c=KC2)
        nc.sync.dma_start(out=W43, in_=W4[:].rearrange("(c p) f -> p c f", p=CH))

        brow = [None] * 4
        ones_rowb = None
        if not zero_bias:
            for i, bt_dram in enumerate([b1, b2, b3, b4]):
                n = D_OUT if i == 3 else D_H
                t = singles.tile([1, n], bf16, tag=f"b{i+1}", name=f"brow{i+1}")
                nc.scalar.dma_start(out=t[:], in_=bt_dram[:])
                brow[i] = t
            ones_colb = singles.tile([1, 128], bf16, tag="onesb")
            nc.vector.tensor_copy(ones_colb[:], ones_col[:])
            ones_rowb = singles.tile([1, BPC], bf16, tag="onesrow")
            nc.vector.memset(ones_rowb[:], 1.0)

        if masked:
            # preload the sigmoid spline table while DMAs stream (the first
            # real sigmoid otherwise pays ~1.3us mid-phase)
            dummy_y = sc_pool.tile([128, 1], bf16, tag="dummy_y")
            nc.scalar.activation(dummy_y[:], junk1[:], AF.Sigmoid)

        def mm_layer(lhs_chunks, w3d, brow_t, nfree, kc):
            """emit matmuls; returns psum tiles [128,512] per batch tile"""
            ps = []
            for bt in range(NBT):
                p = ps_mm.tile([128, 512], f32, tag="mm")
                for kk in range(kc):
                    last = (kk == kc - 1) and (brow_t is None)
                    nc.tensor.matmul(
                        p[:, :nfree],
                        lhs_chunks(kk, bt),
                        w3d[:, kk, :nfree],
                        start=(kk == 0), stop=last)
                if brow_t is not None:
                    nc.tensor.matmul(p[:, :nfree],
                                     ones_colb[:1, :128],
                                     brow_t[:1, :nfree],
                                     start=False, stop=True)
                ps.append(p)
            return ps

        def solve_and_mask(a_ps, layer):
            """a_ps: psum tiles [128,512](:D_H) pre-relu.  Returns am tiles
            [128, D_H] bf16 in SBUF (masked activations)."""
            if not masked:
                am_tiles = []
                for bt in range(NBT):
                    am = am_pool.tile([128, D_H], bf16, tag="am")
                    nc.vector.tensor_scalar(am[:], a_ps[bt][:, :D_H], 0.0,
                                            None, op0=OP.max)
                    am_tiles.append(am)
                return am_tiles

            li = layer - 1
            # --- local rowmax -> gather trigger, ASAP ---
            rm4 = st_pool.tile([128, 4], f32, tag=f"rm4_{li}", name=f"rm4_{li}")
            for bt in range(NBT):
                nc.vector.reduce_max(rm4[:, bt:bt + 1], a_ps[bt][:, :D_H],
                                     axis=AX.X)
            mall = st_pool.tile([128, 1], f32, tag=f"mall{li}")
            nc.vector.reduce_max(mall[:], rm4[:], axis=AX.X)
            nc.vector.tensor_scalar(mall[:], mall[:], 0.0, None, op0=OP.max)
            # vector AllGather: each core contributes its 128 per-partition
            # maxes (skips the PE partition-reduce on the trigger path)
            cc_in = dram.tile([1, 128], f32, tag="ccin")
            cc_out = dram.tile([1, NCORES * 128], f32, tag="ccout")
            nc.gpsimd.dma_start(out=cc_in[:], in_=mall[:, 0:1])
            nc.gpsimd.collective_compute(
                "AllGather", OP.bypass,
                replica_groups=[list(range(NCORES))],
                ins=[cc_in[:]], outs=[cc_out[:]])

            # --- predicted-Cmax (runs during the gather) ---
            # cmaxp = max(1, PRED2*M_loc^2); c1p = -20/cmaxp; c2p = 10/cmaxp
            # layer 1: shard-scalar predictor (PE partition-reduce; plenty of
            # slack before the first gather completes).  layers 2-3:
            # per-partition predictor -- ACT scale/bias are per-partition
            # operands anyway, and the post-gather lambda rescale is exact.
            if li == 0:
                pst = ps_sm.tile([1, 128], f32, tag="pmax")
                nc.tensor.transpose(pst[:1, :128], mall[:, :1], identf[:])
                locmax = sc_pool.tile([1, 1], f32, tag="locmax")
                nc.vector.reduce_max(locmax[:], pst[:1, :128], axis=AX.X)
                c1p3 = sc_pool.tile([1, 3], f32, tag="c1p3", name="c1p3")
                nc.vector.scalar_tensor_tensor(
                    c1p3[:, 2:3], locmax[:], PRED2_SHARD, locmax[:],
                    op0=OP.mult, op1=OP.mult)
                nc.vector.tensor_scalar(c1p3[:, 2:3], c1p3[:, 2:3], 1.0, None,
                                        op0=OP.max)
                rcp = sc_pool.tile([1, 1], f32, tag="rcp")
                nc.vector.reciprocal(rcp[:], c1p3[:, 2:3])
                nc.vector.tensor_scalar(c1p3[:, 0:1], rcp[:], -20.0, None,
                                        op0=OP.mult)
                nc.vector.tensor_scalar(c1p3[:, 1:2], rcp[:], 10.0, None,
                                        op0=OP.mult)
                ps_bp = ps_sm.tile([128, 3], f32, tag="bcast")
                nc.tensor.matmul(ps_bp[:, :3], ones_col[:1, :128],
                                 c1p3[:1, :3], start=True, stop=True)
                cbp = st_pool.tile([128, 3], f32, tag=f"cbp{li}")
                nc.vector.tensor_copy(cbp[:], ps_bp[:, :3])
                c1p_col, c2p_col, cmaxp_col = (cbp[:, 0:1], cbp[:, 1:2],
                                               cbp[:, 2:3])
            else:
                cbp = st_pool.tile([128, 3], f32, tag=f"cbp{li}")
                nc.vector.scalar_tensor_tensor(
                    cbp[:, 2:3], mall[:], PRED2_PART, mall[:],
                    op0=OP.mult, op1=OP.mult)
                nc.vector.tensor_scalar(cbp[:, 2:3], cbp[:, 2:3], 1.0, None,
                                        op0=OP.max)
                rcpc = st_pool.tile([128, 1], f32, tag=f"rcpc{li}")
                nc.vector.reciprocal(rcpc[:], cbp[:, 2:3])
                nc.vector.tensor_scalar(cbp[:, 0:1], rcpc[:], -20.0, None,
                                        op0=OP.mult)
                nc.vector.tensor_scalar(cbp[:, 1:2], rcpc[:], 10.0, None,
                                        op0=OP.mult)
                c1p_col, c2p_col, cmaxp_col = (cbp[:, 0:1], cbp[:, 1:2],
                                               cbp[:, 2:3])

            # --- packed per-row solver state ---
            B4 = st_pool.tile([128, 4], f32, tag=f"B4_{li}", name=f"B4_{li}")
            for bt in range(NBT):
                nc.vector.tensor_copy(B4[:, bt:bt + 1], c2p_col)
            s04 = st_pool.tile([128, 4], f32, tag=f"s04_{li}", name=f"s04_{li}")
            dneg4 = st_pool.tile([128, 4], f32, tag=f"dn4_{li}")
            dd4 = st_pool.tile([128, 4], f32, tag=f"dd4_{li}")
            rd4 = st_pool.tile([128, 4], f32, tag=f"rd4_{li}")
            u4 = st_pool.tile([128, 4], f32, tag=f"u4_{li}")

            # --- relu copies (fp32 solver input): tiles 0/1 on ACT (Relu is
            # in the same table set as Sigmoid), 2/3 on DVE ---
            a_sb = []
            for bt in range(NBT):
                a = a_pool.tile([128, D_H], f32, tag="a")
                if bt < 2:
                    nc.scalar.activation(a[:], a_ps[bt][:, :D_H], AF.Relu)
                else:
                    nc.vector.tensor_scalar(a[:], a_ps[bt][:, :D_H], 0.0,
                                            None, op0=OP.max)
                a_sb.append(a)

            def deriv_engine(bt):
                # (gpsimd rejects scalar_tensor_tensor at BIR verify; keep DVE)
                return nc.vector

            def newton_round(scale_ap, t, per_tile=False):
                """one Newton round; per_tile=True updates B per batch tile so
                the next ACT pass (the finals) can start without a barrier"""
                for bt in range(NBT):
                    y = yb_pool.tile([128, D_H], bf16, tag="yb")
                    nc.scalar.activation(y[:], a_sb[bt][:], AF.Sigmoid,
                                         bias=B4[:, bt:bt + 1], scale=scale_ap,
                                         accum_out=s04[:, bt:bt + 1])
                    t2 = yb_pool.tile([128, D_H], bf16, tag="y2")
                    eng = deriv_engine(bt)
                    eng.scalar_tensor_tensor(
                        t2[:], y[:], 1.0, y[:], op0=OP.subtract, op1=OP.mult,
                        accum_out=dneg4[:, bt:bt + 1])
                    if per_tile:
                        s_ = slice(bt, bt + 1)
                        nc.vector.tensor_scalar(dd4[:, s_], dneg4[:, s_],
                                                -DMIN, None, op0=OP.min)
                        nc.vector.reciprocal(rd4[:, s_], dd4[:, s_])
                        nc.vector.scalar_tensor_tensor(
                            u4[:, s_], s04[:, s_], K_TOPK, rd4[:, s_],
                            op0=OP.subtract, op1=OP.mult)
                        nc.vector.tensor_scalar(u4[:, s_], u4[:, s_], CAP,
                                                -CAP, op0=OP.min, op1=OP.max)
                        nc.vector.tensor_tensor(B4[:, s_], B4[:, s_],
                                                u4[:, s_], op=OP.add)
                if not per_tile:
                    nc.vector.tensor_scalar(dd4[:], dneg4[:], -DMIN, None,
                                            op0=OP.min)
                    nc.vector.reciprocal(rd4[:], dd4[:])
                    nc.vector.scalar_tensor_tensor(u4[:], s04[:], K_TOPK,
                                                   rd4[:], op0=OP.subtract,
                                                   op1=OP.mult)
                    nc.vector.tensor_scalar(u4[:], u4[:], CAP, -CAP,
                                            op0=OP.min, op1=OP.max)
                    nc.vector.tensor_tensor(B4[:], B4[:], u4[:], op=OP.add)
                # dependency-chained dummy matmul: keeps the PE HAM clock
                # warm through the solver so the next burst starts fast
                wp = ps_sm.tile([1, 512], f32, tag="warm")
                nc.tensor.matmul(wp[:1, :64], s04[:, t % 4:t % 4 + 1],
                                 a_sb[t % 4][:, :64], start=True, stop=True)

            def pstate_ramp_chain(n):
                """back-to-back junk streams: the PE p-state only reaches full
                clock after ~3us of CONTINUOUS busy (the L1 burst measured
                383ns/500-col matmul vs 622ns for bursts that start cold).
                Queue-ordered after the solver's last dummy, these keep the PE
                continuously busy through the final rounds so the transpose +
                matmul burst runs at full clock."""
                for _ in range(n):
                    wp = ps_sm.tile([1, 512], f32, tag="warm")
                    nc.tensor.matmul(wp[:1, :512], identb[:, 0:1],
                                     xT3[:, 0, :], start=True, stop=True)

            for t in range(PRE_ROUNDS[li]):
                newton_round(c1p_col, t)

            # --- gather result -> global c1 (bcast), lambda = cmaxp/cmax ---
            g8 = sc_pool.tile([1, NCORES * 128], f32, tag=f"g8_{li}")
            nc.sync.dma_start(out=g8[:], in_=cc_out[:])
            M = sc_pool.tile([1, 1], f32, tag=f"M{li}")
            nc.vector.reduce_max(M[:], g8[:], axis=AX.X)
            rc = sc_pool.tile([1, 1], f32, tag=f"rc{li}")
            nc.vector.tensor_tensor(rc[:], M[:], M[:], op=OP.mult)
            nc.vector.tensor_scalar(rc[:], rc[:], 1.0, None, op0=OP.max)
            nc.vector.reciprocal(rc[:], rc[:])
            ps_bg = ps_sm.tile([128, 3], f32, tag="bcast")
            nc.tensor.matmul(ps_bg[:, :1], ones_col[:1, :128], rc[:1, :1],
                             start=True, stop=True)
            cbg = st_pool.tile([128, 2], f32, tag=f"cbg{li}")
            nc.vector.tensor_scalar(cbg[:, 0:1], ps_bg[:, 0:1], -20.0, None,
                                    op0=OP.mult)
            nc.vector.tensor_tensor(cbg[:, 1:2], cmaxp_col, ps_bg[:, 0:1],
                                    op=OP.mult)
            # warm-start rescale: B *= lambda
            nc.vector.scalar_tensor_tensor(B4[:], B4[:], cbg[:, 1:2], zero4[:],
                                           op0=OP.mult, op1=OP.add)

            for t in range(UPD_ROUNDS[li]):
                newton_round(cbg[:, 0:1], t + 1,
                             per_tile=(t == UPD_ROUNDS[li] - 1))
            pstate_ramp_chain(10)

            # --- final fp32 eval + mask apply, pipelined per tile ---
            am_tiles = []
            for bt in range(NBT):
                yf = yf_pool.tile([128, D_H], f32, tag="yf")
                nc.scalar.activation(yf[:], a_sb[bt][:], AF.Sigmoid,
                                     bias=B4[:, bt:bt + 1], scale=cbg[:, 0:1],
                                     accum_out=s04[:, bt:bt + 1])
                rs = st_pool.tile([128, 1], f32, tag=f"rs{bt}")
                nc.vector.reciprocal(rs[:], s04[:, bt:bt + 1])
                rsk = st_pool.tile([128, 1], f32, tag=f"rsk{bt}")
                nc.vector.tensor_scalar(rsk[:], rs[:], K_TOPK, None, op0=OP.mult)
                am = am_pool.tile([128, D_H], bf16, tag="am")
                nc.vector.scalar_tensor_tensor(
                    am[:], yf[:], rsk[:, 0:1], a_sb[bt][:],
                    op0=OP.mult, op1=OP.mult)
                am_tiles.append(am)
            return am_tiles

        def transpose_act(am_tiles):
            """[128,500] x4 batch tiles -> amT [125, KC2, 512] bf16"""
            amT = amt_pool.tile([CH, KC2 * BPC], bf16, tag="amT")
            amT3 = amT[:].rearrange("p (c f) -> p c f", c=KC2)
            for bt in range(NBT):
                p = ps_tr.tile([128, KC2 * 128], bf16, tag="tr")
                p3 = p[:].rearrange("p (c f) -> p c f", c=KC2)
                for nck in range(KC2):
                    nc.tensor.transpose(
                        p3[:CH, nck, :],
                        am_tiles[bt][:, nck * CH:(nck + 1) * CH],
                        identb[:])
                dst = amT3[:, :, bt * 128:(bt + 1) * 128]
                if bt % 2 == 0:
                    nc.scalar.copy(dst, p3[:CH, :, :])
                else:
                    nc.vector.tensor_copy(dst, p3[:CH, :, :])
            return amT3

        # ================= the network =================
        def l1_lhs(kk, bt):
            return xT3[:, kk, bt * 128:(bt + 1) * 128]

        a_ps = mm_layer(l1_lhs, W13, brow[0], D_H, KC1)
        am1 = solve_and_mask(a_ps, 1)
        am1T = transpose_act(am1)

        def l2_lhs(kk, bt):
            return am1T[:, kk, bt * 128:(bt + 1) * 128]

        a_ps = mm_layer(l2_lhs, W23, brow[1], D_H, KC2)
        am2 = solve_and_mask(a_ps, 2)
        am2T = transpose_act(am2)

        def l3_lhs(kk, bt):
            return am2T[:, kk, bt * 128:(bt + 1) * 128]

        a_ps = mm_layer(l3_lhs, W33, brow[2], D_H, KC2)
        am3 = solve_and_mask(a_ps, 3)
        am3T = transpose_act(am3)

        # ---- layer 4, transposed: out' [10, 512] = sum_k W4k^T @ am3T_k ----
        po = ps_mm.tile([128, 512], f32, tag="mm")
        for kk in range(KC2):
            nc.tensor.matmul(po[:D_OUT, :BPC], W43[:, kk, :D_OUT],
                             am3T[:, kk, :],
                             start=(kk == 0),
                             stop=(kk == KC2 - 1) and (brow[3] is None))
        if brow[3] is not None:
            nc.tensor.matmul(po[:D_OUT, :BPC], brow[3][:1, :D_OUT],
                             ones_rowb[:1, :BPC], start=False, stop=True)
        out_sb = singles.tile([D_OUT, BPC], f32, tag="osb")
        nc.vector.tensor_copy(out_sb[:], po[:D_OUT, :BPC])
        nc.sync.dma_start(out=out[:], in_=out_sb[:])

    nc.compile()
    return nc


def _get_nc(masked: bool, zero_bias: bool = False):
    key = (masked, zero_bias)
    if key not in _CACHE:
        _CACHE[key] = _build(masked, zero_bias)
    return _CACHE[key]


def build_in_maps(x, W1, b1, W2, b2, W3, b3, W4, b4):
    import ml_dtypes
    bf = ml_dtypes.bfloat16
    x = np.asarray(x, np.float32)
    common = {
        "W1": np.ascontiguousarray(np.asarray(W1, np.float32).astype(bf)),
        "W2": np.ascontiguousarray(np.asarray(W2, np.float32).astype(bf)),
        "W3": np.ascontiguousarray(np.asarray(W3, np.float32).astype(bf)),
        "W4": np.ascontiguousarray(np.asarray(W4, np.float32).astype(bf)),
        "b1": np.asarray(b1, np.float32).reshape(1, D_H).astype(bf),
        "b2": np.asarray(b2, np.float32).reshape(1, D_H).astype(bf),
        "b3": np.asarray(b3, np.float32).reshape(1, D_H).astype(bf),
        "b4": np.asarray(b4, np.float32).reshape(1, D_OUT).astype(bf),
    }
    in_maps = []
    for c in range(NCORES):
        xs = x[c * BPC:(c + 1) * BPC, :]
        in_maps.append(
            {"xT": np.ascontiguousarray(xs.T.astype(bf)), **common})
    return in_maps


def kernel(x, W1, b1, W2, b2, W3, b3, W4, b4, sparse):
    s = float(np.asarray(sparse))
    assert s in (0.0, 1.0), f"sparse must be 0 or 1, got {s}"
    zb = all(not np.any(np.asarray(b)) for b in (b1, b2, b3, b4))
    nc = _get_nc(masked=(s == 1.0), zero_bias=zb)

    in_maps = build_in_maps(x, W1, b1, W2, b2, W3, b3, W4, b4)
    from concourse.bass_utils import run_bass_kernel_spmd
    res = run_bass_kernel_spmd(nc, in_maps, core_ids=list(range(NCORES)))
    return np.concatenate(
        [np.ascontiguousarray(res.results[c]["out"].T) for c in range(NCORES)],
        axis=0)


if __name__ == "__main__":
    rng = np.random.default_rng(0)
    ins = {
        "x": rng.standard_normal((BS, D_IN), np.float32),
        "W1": rng.standard_normal((D_IN, D_H), np.float32) / np.sqrt(D_IN),
        "b1": np.zeros(D_H, np.float32),
        "W2": rng.standard_normal((D_H, D_H), np.float32) / np.sqrt(D_H),
        "b2": np.zeros(D_H, np.float32),
        "W3": rng.standard_normal((D_H, D_H), np.float32) / np.sqrt(D_H),
        "b3": np.zeros(D_H, np.float32),
        "W4": rng.standard_normal((D_H, D_OUT), np.float32) / np.sqrt(D_H),
        "b4": np.zeros(D_OUT, np.float32),
        "sparse": 1,
    }
    o = kernel(**ins)
    print("out", o.shape, o.dtype, np.abs(o).max())


# revision 17
# speedup vs baseline: 1.0631x; 1.0631x over previous
"""Trainium2 Bass kernel for nn_NeuralNet_62045097558546 (topk_masking).

Network (fp32): 4-layer MLP with SOFT top-k (Sinkhorn) masking after the
first three ReLU layers.  x:[4096,1024] @ W1:[1024,500] -> mask -> @W2[500,500]
-> mask -> @W3[500,500] -> mask -> @W4[500,10].

Math: the reference's 50 Sinkhorn iterations over anchors {0,1} reduce to a
per-row scalar fixed point: solve sum_j sigmoid(c1*a_j + B) = k for the
per-row ACT bias B, where c1 = -20/Cmax and Cmax = max(M^2, 1) with M the
max activation over the FULL batch (one 8-core AllGather of a scalar per
layer).  mask = (k/s0) * sigmoid(c1*a + B).

Schedule (the point of this version): the first collective cannot complete
before ~75us (ncfw entry barrier + first-op setup), and each later AllGather
takes ~6us.  We hide Newton iterations inside those windows by PRESOLVING
with a predicted Cmax_pre = (1.13*local_max)^2 (engines are otherwise idle
while the gather is in flight), then warm-start the global solve with
B *= c1_glob/c1_pre and run 2 guarded Newton update rounds + a final fp32
sigmoid eval that directly yields the mask.  Matmul path (x, W, masked
activations, PE transposes) runs in bf16 (2x PE stream rate, half DMA);
solver input activations stay fp32.  ReLU is done on DVE/GpSimd max-copies
so ACT keeps its sigmoid table loaded (one table load, in the DMA shadow).
Layer 4 is computed transposed (out' = W4^T-chunks x am3T, free dim 512
instead of 10); the host transposes the [10,512] result back.

Validated on CPU sim vs the 50-iteration reference: rel err ~9e-3 sim-units
(sim overestimates: the same sim scores the previous 197us kernel's schedule
at ~4e-3 while it measures 7e-4 on HW).
"""

import numpy as np
from contextlib import ExitStack

BS, D_IN, D_H, D_OUT = 4096, 1024, 500, 10
NCORES = 8
BPC = BS // NCORES            # 512 batch rows per core
NBT = BPC // 128              # 4 batch tiles of 128
KC1 = D_IN // 128             # 8 contraction chunks for layer 1
CH = 125                      # contraction chunk size for 500-dim layers
KC2 = D_H // CH               # 4 chunks
K_TOPK = 400.0
PRED2_SHARD = 1.28            # (global/shard-max)^2 predictor, layer 1
PRED2_PART = 1.6              # (global/partition-max)^2 predictor, layers 2-3
PRE_ROUNDS = [3, 2, 2]        # local-Cmax presolve rounds per layer
UPD_ROUNDS = [1, 2, 2]        # global-Cmax Newton update rounds per layer
DMIN = 2.0                    # |d| floor (negated-d convention)
CAP = 8.0                     # Newton step clamp

_CACHE = {}


def _build(masked: bool, zero_bias: bool = False):
    import concourse.bass as bass
    import concourse.bacc as bacc
    import concourse.mybir as mybir
    import concourse.tile as tile
    from concourse import masks as cmasks

    f32 = mybir.dt.float32
    bf16 = mybir.dt.bfloat16
    AX = mybir.AxisListType
    OP = mybir.AluOpType
    AF = mybir.ActivationFunctionType

    nc = bacc.Bacc("TRN2", target_bir_lowering=False, debug=False,
                   num_devices=NCORES)

    xT = nc.dram_tensor("xT", [D_IN, BPC], bf16, kind="ExternalInput")
    W1 = nc.dram_tensor("W1", [D_IN, D_H], bf16, kind="ExternalInput")
    W2 = nc.dram_tensor("W2", [D_H, D_H], bf16, kind="ExternalInput")
    W3 = nc.dram_tensor("W3", [D_H, D_H], bf16, kind="ExternalInput")
    W4 = nc.dram_tensor("W4", [D_H, D_OUT], bf16, kind="ExternalInput")
    b1 = nc.dram_tensor("b1", [1, D_H], bf16, kind="ExternalInput")
    b2 = nc.dram_tensor("b2", [1, D_H], bf16, kind="ExternalInput")
    b3 = nc.dram_tensor("b3", [1, D_H], bf16, kind="ExternalInput")
    b4 = nc.dram_tensor("b4", [1, D_OUT], bf16, kind="ExternalInput")
    # transposed output [D_OUT, BPC]; the host transposes back
    out = nc.dram_tensor("out", [D_OUT, BPC], f32, kind="ExternalOutput")

    with tile.TileContext(nc) as tc, ExitStack() as ctx:
        singles = ctx.enter_context(tc.tile_pool(name="singles", bufs=1))
        a_pool = ctx.enter_context(tc.tile_pool(name="a", bufs=NBT + 1))
        yb_pool = ctx.enter_context(tc.tile_pool(name="yb", bufs=3))
        yf_pool = ctx.enter_context(tc.tile_pool(name="yf", bufs=3))
        am_pool = ctx.enter_context(tc.tile_pool(name="am", bufs=NBT))
        amt_pool = ctx.enter_context(tc.tile_pool(name="amt", bufs=2))
        st_pool = ctx.enter_context(tc.tile_pool(name="st", bufs=24))
        sc_pool = ctx.enter_context(tc.tile_pool(name="sc", bufs=2))
        ps_mm = ctx.enter_context(tc.tile_pool(name="ps_mm", bufs=3, space="PSUM"))
        ps_tr = ctx.enter_context(tc.tile_pool(name="ps_tr", bufs=2, space="PSUM"))
        ps_sm = ctx.enter_context(tc.tile_pool(name="ps_sm", bufs=1, space="PSUM"))
        dram = ctx.enter_context(tc.tile_pool(name="dram", bufs=8, space="DRAM"))

        # ---- constants ----
        identf = singles.tile([128, 128], f32, tag="identf")
        cmasks.make_identity(nc, identf[:])
        identb = singles.tile([128, 128], bf16, tag="identb")
        nc.vector.tensor_copy(identb[:], identf[:])
        ones_col = singles.tile([1, 128], f32, tag="ones")
        nc.vector.memset(ones_col[:], 1.0)
        zero4 = singles.tile([128, 4], f32, tag="zero4")
        nc.vector.memset(zero4[:], 0.0)
        junk1 = singles.tile([128, 1], f32, tag="junk1")
        nc.vector.memset(junk1[:], 0.0)

        # ---- weight / input loads (HWDGE), split over two rings ----
        xT_sb = singles.tile([128, KC1 * BPC], bf16, tag="xT")
        xT3 = xT_sb[:].rearrange("p (c f) -> p c f", c=KC1)
        xTd = xT[:].rearrange("(c p) f -> p c f", p=128)
        W1_sb = singles.tile([128, KC1 * D_H], bf16, tag="W1")
        W13 = W1_sb[:].rearrange("p (c f) -> p c f", c=KC1)
        W1d = W1[:].rearrange("(c p) f -> p c f", p=128)
        for kk in range(KC1):
            nc.sync.dma_start(out=xT3[:, kk, :], in_=xTd[:, kk, :])
            nc.scalar.dma_start(out=W13[:, kk, :], in_=W1d[:, kk, :])

        W2_sb = singles.tile([CH, KC2 * D_H], bf16, tag="W2")
        W23 = W2_sb[:].rearrange("p (c f) -> p c f", c=KC2)
        nc.sync.dma_start(out=W23, in_=W2[:].rearrange("(c p) f -> p c f", p=CH))

        W3_sb = singles.tile([CH, KC2 * D_H], bf16, tag="W3")
        W33 = W3_sb[:].rearrange("p (c f) -> p c f", c=KC2)
        nc.scalar.dma_start(out=W33, in_=W3[:].rearrange("(c p) f -> p c f", p=CH))

        W4_sb = singles.tile([CH, KC2 * D_OUT], bf16, tag="W4")
        W43 = W4_sb[:].rearrange("p (c f) -> p c f", c=KC2)
        nc.sync.dma_start(out=W43, in_=W4[:].rearrange("(c p) f -> p c f", p=CH))

        brow = [None] * 4
        ones_rowb = None
        if not zero_bias:
            for i, bt_dram in enumerate([b1, b2, b3, b4]):
                n = D_OUT if i == 3 else D_H
                t = singles.tile([1, n], bf16, tag=f"b{i+1}", name=f"brow{i+1}")
                nc.scalar.dma_start(out=t[:], in_=bt_dram[:])
                brow[i] = t
            ones_colb = singles.tile([1, 128], bf16, tag="onesb")
            nc.vector.tensor_copy(ones_colb[:], ones_col[:])
            ones_rowb = singles.tile([1, BPC], bf16, tag="onesrow")
            nc.vector.memset(ones_rowb[:], 1.0)

        if masked:
            # preload the sigmoid spline table while DMAs stream (the first
            # real sigmoid otherwise pays ~1.3us mid-phase)
            dummy_y = sc_pool.tile([128, 1], bf16, tag="dummy_y")
            nc.scalar.activation(dummy_y[:], junk1[:], AF.Sigmoid)

        def mm_layer(lhs_chunks, w3d, brow_t, nfree, kc):
            """emit matmuls; returns psum tiles [128,512] per batch tile"""
            ps = []
            for bt in range(NBT):
                p = ps_mm.tile([128, 512], f32, tag="mm")
                for kk in range(kc):
                    last = (kk == kc - 1) and (brow_t is None)
                    nc.tensor.matmul(
                        p[:, :nfree],
                        lhs_chunks(kk, bt),
                        w3d[:, kk, :nfree],
                        start=(kk == 0), stop=last)
                if brow_t is not None:
                    nc.tensor.matmul(p[:, :nfree],
                                     ones_colb[:1, :128],
                                     brow_t[:1, :nfree],
                                     start=False, stop=True)
                ps.append(p)
            return ps

        def solve_and_mask(a_ps, layer):
            """a_ps: psum tiles [128,512](:D_H) pre-relu.  Returns am tiles
            [128, D_H] bf16 in SBUF (masked activations)."""
            if not masked:
                am_tiles = []
                for bt in range(NBT):
                    am = am_pool.tile([128, D_H], bf16, tag="am")
                    nc.vector.tensor_scalar(am[:], a_ps[bt][:, :D_H], 0.0,
                                            None, op0=OP.max)
                    am_tiles.append(am)
                return am_tiles

            li = layer - 1
            # --- local rowmax -> gather trigger, ASAP ---
            rm4 = st_pool.tile([128, 4], f32, tag=f"rm4_{li}", name=f"rm4_{li}")
            for bt in range(NBT):
                nc.vector.reduce_max(rm4[:, bt:bt + 1], a_ps[bt][:, :D_H],
                                     axis=AX.X)
            mall = st_pool.tile([128, 1], f32, tag=f"mall{li}")
            nc.vector.reduce_max(mall[:], rm4[:], axis=AX.X)
            nc.vector.tensor_scalar(mall[:], mall[:], 0.0, None, op0=OP.max)
            # scalar AllGather (a [128,1]-strided cc_in DMA pays ~90ns/element
            # descriptor overhead -> 12us; one 4B element is fast)
            pst = ps_sm.tile([1, 128], f32, tag="pmax")
            nc.tensor.transpose(pst[:1, :128], mall[:, :1], identf[:])
            locmax = sc_pool.tile([1, 1], f32, tag=f"locmax{li}",
                                  name=f"locmax{li}")
            nc.vector.reduce_max(locmax[:], pst[:1, :128], axis=AX.X)
            cc_in = dram.tile([1, 1], f32, tag="ccin")
            cc_out = dram.tile([1, NCORES], f32, tag="ccout")
            nc.gpsimd.dma_start(out=cc_in[:], in_=locmax[:])
            nc.gpsimd.collective_compute(
                "AllGather", OP.bypass,
                replica_groups=[list(range(NCORES))],
                ins=[cc_in[:]], outs=[cc_out[:]])

            # --- predicted-Cmax (runs during the gather) ---
            # cmaxp = max(1, PRED2*M_loc^2); c1p = -20/cmaxp; c2p = 10/cmaxp
            # layer 1: shard-scalar predictor (PE partition-reduce; plenty of
            # slack before the first gather completes).  layers 2-3:
            # per-partition predictor -- ACT scale/bias are per-partition
            # operands anyway, and the post-gather lambda rescale is exact.
            if li == 0:
                c1p3 = sc_pool.tile([1, 3], f32, tag="c1p3", name="c1p3")
                nc.vector.scalar_tensor_tensor(
                    c1p3[:, 2:3], locmax[:], PRED2_SHARD, locmax[:],
                    op0=OP.mult, op1=OP.mult)
                nc.vector.tensor_scalar(c1p3[:, 2:3], c1p3[:, 2:3], 1.0, None,
                                        op0=OP.max)
                rcp = sc_pool.tile([1, 1], f32, tag="rcp")
                nc.vector.reciprocal(rcp[:], c1p3[:, 2:3])
                nc.vector.tensor_scalar(c1p3[:, 0:1], rcp[:], -20.0, None,
                                        op0=OP.mult)
                nc.vector.tensor_scalar(c1p3[:, 1:2], rcp[:], 10.0, None,
                                        op0=OP.mult)
                ps_bp = ps_sm.tile([128, 3], f32, tag="bcast")
                nc.tensor.matmul(ps_bp[:, :3], ones_col[:1, :128],
                                 c1p3[:1, :3], start=True, stop=True)
                cbp = st_pool.tile([128, 3], f32, tag=f"cbp{li}")
                nc.vector.tensor_copy(cbp[:], ps_bp[:, :3])
                c1p_col, c2p_col, cmaxp_col = (cbp[:, 0:1], cbp[:, 1:2],
                                               cbp[:, 2:3])
            else:
                cbp = st_pool.tile([128, 3], f32, tag=f"cbp{li}")
                nc.vector.scalar_tensor_tensor(
                    cbp[:, 2:3], mall[:], PRED2_PART, mall[:],
                    op0=OP.mult, op1=OP.mult)
                nc.vector.tensor_scalar(cbp[:, 2:3], cbp[:, 2:3], 1.0, None,
                                        op0=OP.max)
                rcpc = st_pool.tile([128, 1], f32, tag=f"rcpc{li}")
                nc.vector.reciprocal(rcpc[:], cbp[:, 2:3])
                nc.vector.tensor_scalar(cbp[:, 0:1], rcpc[:], -20.0, None,
                                        op0=OP.mult)
                nc.vector.tensor_scalar(cbp[:, 1:2], rcpc[:], 10.0, None,
                                        op0=OP.mult)
                c1p_col, c2p_col, cmaxp_col = (cbp[:, 0:1], cbp[:, 1:2],
                                               cbp[:, 2:3])

            # --- packed per-row solver state ---
            B4 = st_pool.tile([128, 4], f32, tag=f"B4_{li}", name=f"B4_{li}")
            for bt in range(NBT):
                nc.vector.tensor_copy(B4[:, bt:bt + 1], c2p_col)
            s04 = st_pool.tile([128, 4], f32, tag=f"s04_{li}", name=f"s04_{li}")
            dneg4 = st_pool.tile([128, 4], f32, tag=f"dn4_{li}")
            dd4 = st_pool.tile([128, 4], f32, tag=f"dd4_{li}")
            rd4 = st_pool.tile([128, 4], f32, tag=f"rd4_{li}")
            u4 = st_pool.tile([128, 4], f32, tag=f"u4_{li}")

            # --- relu copies (fp32 solver input): tiles 0/1 on ACT (Relu is
            # in the same table set as Sigmoid), 2/3 on DVE ---
            a_sb = []
            for bt in range(NBT):
                a = a_pool.tile([128, D_H], f32, tag="a")
                if bt < 2:
                    nc.scalar.activation(a[:], a_ps[bt][:, :D_H], AF.Relu)
                else:
                    nc.vector.tensor_scalar(a[:], a_ps[bt][:, :D_H], 0.0,
                                            None, op0=OP.max)
                a_sb.append(a)

            def deriv_engine(bt):
                # (gpsimd rejects scalar_tensor_tensor at BIR verify; keep DVE)
                return nc.vector

            def newton_round(scale_ap, t, per_tile=False):
                """one Newton round; per_tile=True updates B per batch tile so
                the next ACT pass (the finals) can start without a barrier"""
                for bt in range(NBT):
                    y = yb_pool.tile([128, D_H], bf16, tag="yb")
                    nc.scalar.activation(y[:], a_sb[bt][:], AF.Sigmoid,
                                         bias=B4[:, bt:bt + 1], scale=scale_ap,
                                         accum_out=s04[:, bt:bt + 1])
                    t2 = yb_pool.tile([128, D_H], bf16, tag="y2")
                    eng = deriv_engine(bt)
                    eng.scalar_tensor_tensor(
                        t2[:], y[:], 1.0, y[:], op0=OP.subtract, op1=OP.mult,
                        accum_out=dneg4[:, bt:bt + 1])
                    if per_tile:
                        s_ = slice(bt, bt + 1)
                        nc.vector.tensor_scalar(dd4[:, s_], dneg4[:, s_],
                                                -DMIN, None, op0=OP.min)
                        nc.vector.reciprocal(rd4[:, s_], dd4[:, s_])
                        nc.vector.scalar_tensor_tensor(
                            u4[:, s_], s04[:, s_], K_TOPK, rd4[:, s_],
                            op0=OP.subtract, op1=OP.mult)
                        nc.vector.tensor_scalar(u4[:, s_], u4[:, s_], CAP,
                                                -CAP, op0=OP.min, op1=OP.max)
                        nc.vector.tensor_tensor(B4[:, s_], B4[:, s_],
                                                u4[:, s_], op=OP.add)
                if not per_tile:
                    nc.vector.tensor_scalar(dd4[:], dneg4[:], -DMIN, None,
                                            op0=OP.min)
                    nc.vector.reciprocal(rd4[:], dd4[:])
                    nc.vector.scalar_tensor_tensor(u4[:], s04[:], K_TOPK,
                                                   rd4[:], op0=OP.subtract,
                                                   op1=OP.mult)
                    nc.vector.tensor_scalar(u4[:], u4[:], CAP, -CAP,
                                            op0=OP.min, op1=OP.max)
                    nc.vector.tensor_tensor(B4[:], B4[:], u4[:], op=OP.add)
                # dependency-chained dummy matmul: keeps the PE HAM clock
                # warm through the solver so the next burst starts fast
                wp = ps_sm.tile([1, 512], f32, tag="warm")
                nc.tensor.matmul(wp[:1, :64], s04[:, t % 4:t % 4 + 1],
                                 a_sb[t % 4][:, :64], start=True, stop=True)

            def pstate_ramp_chain(n):
                """back-to-back junk streams: the PE p-state only reaches full
                clock after ~3us of CONTINUOUS busy (the L1 burst measured
                383ns/500-col matmul vs 622ns for bursts that start cold).
                Queue-ordered after the solver's last dummy, these keep the PE
                continuously busy through the final rounds so the transpose +
                matmul burst runs at full clock."""
                for _ in range(n):
                    wp = ps_sm.tile([1, 512], f32, tag="warm")
                    nc.tensor.matmul(wp[:1, :512], identb[:, 0:1],
                                     xT3[:, 0, :], start=True, stop=True)

            for t in range(PRE_ROUNDS[li]):
                newton_round(c1p_col, t)

            # --- gather result -> global c1 (bcast), lambda = cmaxp/cmax ---
            g8 = sc_pool.tile([1, NCORES], f32, tag=f"g8_{li}")
            nc.sync.dma_start(out=g8[:], in_=cc_out[:])
            M = sc_pool.tile([1, 1], f32, tag=f"M{li}")
            nc.vector.reduce_max(M[:], g8[:], axis=AX.X)
            rc = sc_pool.tile([1, 1], f32, tag=f"rc{li}")
            nc.vector.tensor_tensor(rc[:], M[:], M[:], op=OP.mult)
            nc.vector.tensor_scalar(rc[:], rc[:], 1.0, None, op0=OP.max)
            nc.vector.reciprocal(rc[:], rc[:])
            ps_bg = ps_sm.tile([128, 3], f32, tag="bcast")
            nc.tensor.matmul(ps_bg[:, :1], ones_col[:1, :128], rc[:1, :1],
                             start=True, stop=True)
            cbg = st_pool.tile([128, 2], f32, tag=f"cbg{li}")
            nc.vector.tensor_scalar(cbg[:, 0:1], ps_bg[:, 0:1], -20.0, None,
                                    op0=OP.mult)
            nc.vector.tensor_tensor(cbg[:, 1:2], cmaxp_col, ps_bg[:, 0:1],
                                    op=OP.mult)
            # warm-start rescale: B *= lambda
            nc.vector.scalar_tensor_tensor(B4[:], B4[:], cbg[:, 1:2], zero4[:],
                                           op0=OP.mult, op1=OP.add)

            for t in range(UPD_ROUNDS[li]):
                last = (t == UPD_ROUNDS[li] - 1)
                newton_round(cbg[:, 0:1], t + 1, per_tile=last)
                # p-state ramp: ~10us of continuous PE busy is needed before
                # the transpose+matmul burst runs at full clock; fill the
                # solver tail without running past the start of the burst
                pstate_ramp_chain(3 if last else 5)

            # --- final fp32 eval + mask apply, pipelined per tile ---
            am_tiles = []
            for bt in range(NBT):
                yf = yf_pool.tile([128, D_H], f32, tag="yf")
                nc.scalar.activation(yf[:], a_sb[bt][:], AF.Sigmoid,
                                     bias=B4[:, bt:bt + 1], scale=cbg[:, 0:1],
                                     accum_out=s04[:, bt:bt + 1])
                rs = st_pool.tile([128, 1], f32, tag=f"rs{bt}")
                nc.vector.reciprocal(rs[:], s04[:, bt:bt + 1])
                rsk = st_pool.tile([128, 1], f32, tag=f"rsk{bt}")
                nc.vector.tensor_scalar(rsk[:], rs[:], K_TOPK, None, op0=OP.mult)
                am = am_pool.tile([128, D_H], bf16, tag="am")
                nc.vector.scalar_tensor_tensor(
                    am[:], yf[:], rsk[:, 0:1], a_sb[bt][:],
                    op0=OP.mult, op1=OP.mult)
                am_tiles.append(am)
            return am_tiles

        def transpose_act(am_tiles):
            """[128,500] x4 batch tiles -> amT [125, KC2, 512] bf16"""
            amT = amt_pool.tile([CH, KC2 * BPC], bf16, tag="amT")
            amT3 = amT[:].rearrange("p (c f) -> p c f", c=KC2)
            for bt in range(NBT):
                p = ps_tr.tile([128, KC2 * 128], bf16, tag="tr")
                p3 = p[:].rearrange("p (c f) -> p c f", c=KC2)
                for nck in range(KC2):
                    nc.tensor.transpose(
                        p3[:CH, nck, :],
                        am_tiles[bt][:, nck * CH:(nck + 1) * CH],
                        identb[:])
                dst = amT3[:, :, bt * 128:(bt + 1) * 128]
                if bt % 2 == 0:
                    nc.scalar.copy(dst, p3[:CH, :, :])
                else:
                    nc.vector.tensor_copy(dst, p3[:CH, :, :])
            return amT3

        # ================= the network =================
        def l1_lhs(kk, bt):
            return xT3[:, kk, bt * 128:(bt + 1) * 128]

        a_ps = mm_layer(l1_lhs, W13, brow[0], D_H, KC1)
        am1 = solve_and_mask(a_ps, 1)
        am1T = transpose_act(am1)

        def l2_lhs(kk, bt):
            return am1T[:, kk, bt * 128:(bt + 1) * 128]

        a_ps = mm_layer(l2_lhs, W23, brow[1], D_H, KC2)
        am2 = solve_and_mask(a_ps, 2)
        am2T = transpose_act(am2)

        def l3_lhs(kk, bt):
            return am2T[:, kk, bt * 128:(bt + 1) * 128]

        a_ps = mm_layer(l3_lhs, W33, brow[2], D_H, KC2)
        am3 = solve_and_mask(a_ps, 3)
        am3T = transpose_act(am3)

        # ---- layer 4, transposed: out' [10, 512] = sum_k W4k^T @ am3T_k ----
        po = ps_mm.tile([128, 512], f32, tag="mm")
        for kk in range(KC2):
            nc.tensor.matmul(po[:D_OUT, :BPC], W43[:, kk, :D_OUT],
                             am3T[:, kk, :],
                             start=(kk == 0),
                             stop=(kk == KC2 - 1) and (brow[3] is None))
        if brow[3] is not None:
            nc.tensor.matmul(po[:D_OUT, :BPC], brow[3][:1, :D_OUT],
                             ones_rowb[:1, :BPC], start=False, stop=True)
        out_sb = singles.tile([D_OUT, BPC], f32, tag="osb")
        nc.vector.tensor_copy(out_sb[:], po[:D_OUT, :BPC])
        nc.sync.dma_start(out=out[:], in_=out_sb[:])

    nc.compile()
    return nc


def _get_nc(masked: bool, zero_bias: bool = False):
    key = (masked, zero_bias)
    if key not in _CACHE:
        _CACHE[key] = _build(masked, zero_bias)
    return _CACHE[key]


def build_in_maps(x, W1, b1, W2, b2, W3, b3, W4, b4):
    import ml_dtypes
    bf = ml_dtypes.bfloat16
    x = np.asarray(x, np.float32)
    common = {
        "W1": np.ascontiguousarray(np.asarray(W1, np.float32).astype(bf)),
        "W2": np.ascontiguousarray(np.asarray(W2, np.float32).astype(bf)),
        "W3": np.ascontiguousarray(np.asarray(W3, np.float32).astype(bf)),
        "W4": np.ascontiguousarray(np.asarray(W4, np.float32).astype(bf)),
        "b1": np.asarray(b1, np.float32).reshape(1, D_H).astype(bf),
        "b2": np.asarray(b2, np.float32).reshape(1, D_H).astype(bf),
        "b3": np.asarray(b3, np.float32).reshape(1, D_H).astype(bf),
        "b4": np.asarray(b4, np.float32).reshape(1, D_OUT).astype(bf),
    }
    in_maps = []
    for c in range(NCORES):
        xs = x[c * BPC:(c + 1) * BPC, :]
        in_maps.append(
            {"xT": np.ascontiguousarray(xs.T.astype(bf)), **common})
    return in_maps


def kernel(x, W1, b1, W2, b2, W3, b3, W4, b4, sparse):
    s = float(np.asarray(sparse))
    assert s in (0.0, 1.0), f"sparse must be 0 or 1, got {s}"
    zb = all(not np.any(np.asarray(b)) for b in (b1, b2, b3, b4))
    nc = _get_nc(masked=(s == 1.0), zero_bias=zb)

    in_maps = build_in_maps(x, W1, b1, W2, b2, W3, b3, W4, b4)
    from concourse.bass_utils import run_bass_kernel_spmd
    res = run_bass_kernel_spmd(nc, in_maps, core_ids=list(range(NCORES)))
    return np.concatenate(
        [np.ascontiguousarray(res.results[c]["out"].T) for c in range(NCORES)],
        axis=0)


if __name__ == "__main__":
    rng = np.random.default_rng(0)
    ins = {
        "x": rng.standard_normal((BS, D_IN), np.float32),
        "W1": rng.standard_normal((D_IN, D_H), np.float32) / np.sqrt(D_IN),
        "b1": np.zeros(D_H, np.float32),
        "W2": rng.standard_normal((D_H, D_H), np.float32) / np.sqrt(D_H),
        "b2": np.zeros(D_H, np.float32),
        "W3": rng.standard_normal((D_H, D_H), np.float32) / np.sqrt(D_H),
        "b3": np.zeros(D_H, np.float32),
        "W4": rng.standard_normal((D_H, D_OUT), np.float32) / np.sqrt(D_H),
        "b4": np.zeros(D_OUT, np.float32),
        "sparse": 1,
    }
    o = kernel(**ins)
    print("out", o.shape, o.dtype, np.abs(o).max())


# revision 21
# speedup vs baseline: 1.1084x; 1.0426x over previous
"""Trainium2 Bass kernel for nn_NeuralNet_62045097558546 (topk_masking).

Network (fp32): 4-layer MLP with SOFT top-k (Sinkhorn) masking after the
first three ReLU layers.  x:[4096,1024] @ W1:[1024,500] -> mask -> @W2[500,500]
-> mask -> @W3[500,500] -> mask -> @W4[500,10].

Math: the reference's 50 Sinkhorn iterations over anchors {0,1} reduce to a
per-row scalar fixed point: solve sum_j sigmoid(c1*a_j + B) = k for the
per-row ACT bias B, where c1 = -20/Cmax and Cmax = max(M^2, 1) with M the
max activation over the FULL batch (one 8-core AllGather of a scalar per
layer).  mask = (k/s0) * sigmoid(c1*a + B).

Schedule (the point of this version): the first collective cannot complete
before ~75us (ncfw entry barrier + first-op setup), and each later AllGather
takes ~6us.  We hide Newton iterations inside those windows by PRESOLVING
with a predicted Cmax_pre = (1.13*local_max)^2 (engines are otherwise idle
while the gather is in flight), then warm-start the global solve with
B *= c1_glob/c1_pre and run 2 guarded Newton update rounds + a final fp32
sigmoid eval that directly yields the mask.  Matmul path (x, W, masked
activations, PE transposes) runs in bf16 (2x PE stream rate, half DMA);
solver input activations stay fp32.  ReLU is done on DVE/GpSimd max-copies
so ACT keeps its sigmoid table loaded (one table load, in the DMA shadow).
Layer 4 is computed transposed (out' = W4^T-chunks x am3T, free dim 512
instead of 10); the host transposes the [10,512] result back.

Validated on CPU sim vs the 50-iteration reference: rel err ~9e-3 sim-units
(sim overestimates: the same sim scores the previous 197us kernel's schedule
at ~4e-3 while it measures 7e-4 on HW).
"""

import numpy as np
from contextlib import ExitStack

BS, D_IN, D_H, D_OUT = 4096, 1024, 500, 10
NCORES = 8
BPC = BS // NCORES            # 512 batch rows per core
NBT = BPC // 128              # 4 batch tiles of 128
KC1 = D_IN // 128             # 8 contraction chunks for layer 1
CH = 125                      # contraction chunk size for 500-dim layers
KC2 = D_H // CH               # 4 chunks
K_TOPK = 400.0
PRED2_SHARD = 1.28            # (global/shard-max)^2 predictor, layer 1
PRED2_PART = 1.6              # (global/partition-max)^2 predictor, layers 2-3
PRE_ROUNDS = [3, 2, 2]        # local-Cmax presolve rounds per layer
UPD_ROUNDS = [1, 2, 2]        # global-Cmax Newton update rounds per layer
DMIN = 2.0                    # |d| floor (negated-d convention)
CAP = 8.0                     # Newton step clamp

_CACHE = {}


def _build(masked: bool, zero_bias: bool = False):
    import concourse.bass as bass
    import concourse.bacc as bacc
    import concourse.mybir as mybir
    import concourse.tile as tile
    from concourse import masks as cmasks

    f32 = mybir.dt.float32
    bf16 = mybir.dt.bfloat16
    AX = mybir.AxisListType
    OP = mybir.AluOpType
    AF = mybir.ActivationFunctionType

    nc = bacc.Bacc("TRN2", target_bir_lowering=False, debug=False,
                   num_devices=NCORES)

    xT = nc.dram_tensor("xT", [D_IN, BPC], bf16, kind="ExternalInput")
    W1 = nc.dram_tensor("W1", [D_IN, D_H], bf16, kind="ExternalInput")
    W2 = nc.dram_tensor("W2", [D_H, D_H], bf16, kind="ExternalInput")
    W3 = nc.dram_tensor("W3", [D_H, D_H], bf16, kind="ExternalInput")
    W4 = nc.dram_tensor("W4", [D_H, D_OUT], bf16, kind="ExternalInput")
    b1 = nc.dram_tensor("b1", [1, D_H], bf16, kind="ExternalInput")
    b2 = nc.dram_tensor("b2", [1, D_H], bf16, kind="ExternalInput")
    b3 = nc.dram_tensor("b3", [1, D_H], bf16, kind="ExternalInput")
    b4 = nc.dram_tensor("b4", [1, D_OUT], bf16, kind="ExternalInput")
    # transposed output [D_OUT, BPC]; the host transposes back
    out = nc.dram_tensor("out", [D_OUT, BPC], f32, kind="ExternalOutput")

    with tile.TileContext(nc) as tc, ExitStack() as ctx:
        singles = ctx.enter_context(tc.tile_pool(name="singles", bufs=1))
        a_pool = ctx.enter_context(tc.tile_pool(name="a", bufs=NBT + 1))
        yb_pool = ctx.enter_context(tc.tile_pool(name="yb", bufs=3))
        yf_pool = ctx.enter_context(tc.tile_pool(name="yf", bufs=3))
        am_pool = ctx.enter_context(tc.tile_pool(name="am", bufs=NBT))
        amt_pool = ctx.enter_context(tc.tile_pool(name="amt", bufs=2))
        st_pool = ctx.enter_context(tc.tile_pool(name="st", bufs=24))
        sc_pool = ctx.enter_context(tc.tile_pool(name="sc", bufs=2))
        ps_mm = ctx.enter_context(tc.tile_pool(name="ps_mm", bufs=3, space="PSUM"))
        ps_tr = ctx.enter_context(tc.tile_pool(name="ps_tr", bufs=2, space="PSUM"))
        ps_sm = ctx.enter_context(tc.tile_pool(name="ps_sm", bufs=1, space="PSUM"))
        dram = ctx.enter_context(tc.tile_pool(name="dram", bufs=8, space="DRAM"))

        # ---- constants ----
        identf = singles.tile([128, 128], f32, tag="identf")
        cmasks.make_identity(nc, identf[:])
        identb = singles.tile([128, 128], bf16, tag="identb")
        nc.vector.tensor_copy(identb[:], identf[:])
        ones_col = singles.tile([1, 128], f32, tag="ones")
        nc.vector.memset(ones_col[:], 1.0)
        zero4 = singles.tile([128, 4], f32, tag="zero4")
        nc.vector.memset(zero4[:], 0.0)
        junk1 = singles.tile([128, 1], f32, tag="junk1")
        nc.vector.memset(junk1[:], 0.0)

        # ---- weight / input loads (HWDGE), split over two rings ----
        xT_sb = singles.tile([128, KC1 * BPC], bf16, tag="xT")
        xT3 = xT_sb[:].rearrange("p (c f) -> p c f", c=KC1)
        xTd = xT[:].rearrange("(c p) f -> p c f", p=128)
        W1_sb = singles.tile([128, KC1 * D_H], bf16, tag="W1")
        W13 = W1_sb[:].rearrange("p (c f) -> p c f", c=KC1)
        W1d = W1[:].rearrange("(c p) f -> p c f", p=128)
        for kk in range(KC1):
            nc.sync.dma_start(out=xT3[:, kk, :], in_=xTd[:, kk, :])
            nc.scalar.dma_start(out=W13[:, kk, :], in_=W1d[:, kk, :])

        W2_sb = singles.tile([CH, KC2 * D_H], bf16, tag="W2")
        W23 = W2_sb[:].rearrange("p (c f) -> p c f", c=KC2)
        nc.sync.dma_start(out=W23, in_=W2[:].rearrange("(c p) f -> p c f", p=CH))

        W3_sb = singles.tile([CH, KC2 * D_H], bf16, tag="W3")
        W33 = W3_sb[:].rearrange("p (c f) -> p c f", c=KC2)
        nc.scalar.dma_start(out=W33, in_=W3[:].rearrange("(c p) f -> p c f", p=CH))

        W4_sb = singles.tile([CH, KC2 * D_OUT], bf16, tag="W4")
        W43 = W4_sb[:].rearrange("p (c f) -> p c f", c=KC2)
        nc.sync.dma_start(out=W43, in_=W4[:].rearrange("(c p) f -> p c f", p=CH))

        brow = [None] * 4
        ones_rowb = None
        if not zero_bias:
            for i, bt_dram in enumerate([b1, b2, b3, b4]):
                n = D_OUT if i == 3 else D_H
                t = singles.tile([1, n], bf16, tag=f"b{i+1}", name=f"brow{i+1}")
                nc.scalar.dma_start(out=t[:], in_=bt_dram[:])
                brow[i] = t
            ones_colb = singles.tile([1, 128], bf16, tag="onesb")
            nc.vector.tensor_copy(ones_colb[:], ones_col[:])
            ones_rowb = singles.tile([1, BPC], bf16, tag="onesrow")
            nc.vector.memset(ones_rowb[:], 1.0)

        if masked:
            # preload the sigmoid spline table while DMAs stream (the first
            # real sigmoid otherwise pays ~1.3us mid-phase)
            dummy_y = sc_pool.tile([128, 1], bf16, tag="dummy_y")
            nc.scalar.activation(dummy_y[:], junk1[:], AF.Sigmoid)

        def mm_layer(lhs_chunks, w3d, brow_t, nfree, kc):
            """emit matmuls; returns psum tiles [128,512] per batch tile"""
            ps = []
            for bt in range(NBT):
                p = ps_mm.tile([128, 512], f32, tag="mm")
                for kk in range(kc):
                    last = (kk == kc - 1) and (brow_t is None)
                    nc.tensor.matmul(
                        p[:, :nfree],
                        lhs_chunks(kk, bt),
                        w3d[:, kk, :nfree],
                        start=(kk == 0), stop=last)
                if brow_t is not None:
                    nc.tensor.matmul(p[:, :nfree],
                                     ones_colb[:1, :128],
                                     brow_t[:1, :nfree],
                                     start=False, stop=True)
                ps.append(p)
            return ps

        def solve_and_mask(a_ps, layer):
            """a_ps: psum tiles [128,512](:D_H) pre-relu.  Returns am tiles
            [128, D_H] bf16 in SBUF (masked activations)."""
            if not masked:
                am_tiles = []
                for bt in range(NBT):
                    am = am_pool.tile([128, D_H], bf16, tag="am")
                    nc.vector.tensor_scalar(am[:], a_ps[bt][:, :D_H], 0.0,
                                            None, op0=OP.max)
                    am_tiles.append(am)
                return am_tiles

            li = layer - 1
            # --- local rowmax -> gather trigger, ASAP ---
            rm4 = st_pool.tile([128, 4], f32, tag=f"rm4_{li}", name=f"rm4_{li}")
            for bt in range(NBT):
                nc.vector.reduce_max(rm4[:, bt:bt + 1], a_ps[bt][:, :D_H],
                                     axis=AX.X)
            mall = st_pool.tile([128, 1], f32, tag=f"mall{li}")
            nc.vector.reduce_max(mall[:], rm4[:], axis=AX.X)
            nc.vector.tensor_scalar(mall[:], mall[:], 0.0, None, op0=OP.max)
            # scalar AllGather (a [128,1]-strided cc_in DMA pays ~90ns/element
            # descriptor overhead -> 12us; one 4B element is fast)
            pst = ps_sm.tile([1, 128], f32, tag="pmax")
            nc.tensor.transpose(pst[:1, :128], mall[:, :1], identf[:])
            locmax = sc_pool.tile([1, 1], f32, tag=f"locmax{li}",
                                  name=f"locmax{li}")
            nc.vector.reduce_max(locmax[:], pst[:1, :128], axis=AX.X)
            cc_in = dram.tile([1, 1], f32, tag="ccin")
            cc_out = dram.tile([1, NCORES], f32, tag="ccout")
            nc.gpsimd.dma_start(out=cc_in[:], in_=locmax[:])
            nc.gpsimd.collective_compute(
                "AllGather", OP.bypass,
                replica_groups=[list(range(NCORES))],
                ins=[cc_in[:]], outs=[cc_out[:]])

            # --- predicted-Cmax (runs during the gather) ---
            # cmaxp = max(1, PRED2*M_loc^2); c1p = -20/cmaxp; c2p = 10/cmaxp
            # layer 1: shard-scalar predictor (PE partition-reduce; plenty of
            # slack before the first gather completes).  layers 2-3:
            # per-partition predictor -- ACT scale/bias are per-partition
            # operands anyway, and the post-gather lambda rescale is exact.
            if li == 0:
                c1p3 = sc_pool.tile([1, 3], f32, tag="c1p3", name="c1p3")
                nc.vector.scalar_tensor_tensor(
                    c1p3[:, 2:3], locmax[:], PRED2_SHARD, locmax[:],
                    op0=OP.mult, op1=OP.mult)
                nc.vector.tensor_scalar(c1p3[:, 2:3], c1p3[:, 2:3], 1.0, None,
                                        op0=OP.max)
                rcp = sc_pool.tile([1, 1], f32, tag="rcp")
                nc.vector.reciprocal(rcp[:], c1p3[:, 2:3])
                nc.vector.tensor_scalar(c1p3[:, 0:1], rcp[:], -20.0, None,
                                        op0=OP.mult)
                nc.vector.tensor_scalar(c1p3[:, 1:2], rcp[:], 10.0, None,
                                        op0=OP.mult)
                ps_bp = ps_sm.tile([128, 3], f32, tag="bcast")
                nc.tensor.matmul(ps_bp[:, :3], ones_col[:1, :128],
                                 c1p3[:1, :3], start=True, stop=True)
                cbp = st_pool.tile([128, 3], f32, tag=f"cbp{li}")
                nc.vector.tensor_copy(cbp[:], ps_bp[:, :3])
                c1p_col, c2p_col, cmaxp_col = (cbp[:, 0:1], cbp[:, 1:2],
                                               cbp[:, 2:3])
            else:
                cbp = st_pool.tile([128, 3], f32, tag=f"cbp{li}")
                nc.vector.scalar_tensor_tensor(
                    cbp[:, 2:3], mall[:], PRED2_PART, mall[:],
                    op0=OP.mult, op1=OP.mult)
                nc.vector.tensor_scalar(cbp[:, 2:3], cbp[:, 2:3], 1.0, None,
                                        op0=OP.max)
                rcpc = st_pool.tile([128, 1], f32, tag=f"rcpc{li}")
                nc.vector.reciprocal(rcpc[:], cbp[:, 2:3])
                nc.vector.tensor_scalar(cbp[:, 0:1], rcpc[:], -20.0, None,
                                        op0=OP.mult)
                nc.vector.tensor_scalar(cbp[:, 1:2], rcpc[:], 10.0, None,
                                        op0=OP.mult)
                c1p_col, c2p_col, cmaxp_col = (cbp[:, 0:1], cbp[:, 1:2],
                                               cbp[:, 2:3])

            # --- packed per-row solver state ---
            B4 = st_pool.tile([128, 4], f32, tag=f"B4_{li}", name=f"B4_{li}")
            for bt in range(NBT):
                nc.vector.tensor_copy(B4[:, bt:bt + 1], c2p_col)
            s04 = st_pool.tile([128, 4], f32, tag=f"s04_{li}", name=f"s04_{li}")
            dneg4 = st_pool.tile([128, 4], f32, tag=f"dn4_{li}")
            dd4 = st_pool.tile([128, 4], f32, tag=f"dd4_{li}")
            rd4 = st_pool.tile([128, 4], f32, tag=f"rd4_{li}")
            u4 = st_pool.tile([128, 4], f32, tag=f"u4_{li}")

            # --- relu copies (fp32 solver input): tiles 0/1 on ACT (Relu is
            # in the same table set as Sigmoid), 2/3 on DVE ---
            a_sb = []
            for bt in range(NBT):
                a = a_pool.tile([128, D_H], f32, tag="a")
                if bt < 3:
                    nc.scalar.activation(a[:], a_ps[bt][:, :D_H], AF.Relu)
                else:
                    nc.vector.tensor_scalar(a[:], a_ps[bt][:, :D_H], 0.0,
                                            None, op0=OP.max)
                a_sb.append(a)

            def deriv_engine(bt):
                # (gpsimd rejects scalar_tensor_tensor at BIR verify; keep DVE)
                return nc.vector

            s0p4 = st_pool.tile([128, 4], f32, tag=f"s0p4_{li}")
            ru4 = st_pool.tile([128, 4], f32, tag=f"ru4_{li}")

            def newton_round(scale_ap, t, per_tile=False, snapshot=False):
                """one Newton round; per_tile=True updates B per batch tile so
                the next ACT pass can start without a barrier.  snapshot saves
                (s0, 1/u) per tile for a following secant round."""
                for bt in range(NBT):
                    y = yb_pool.tile([128, D_H], bf16, tag="yb")
                    nc.scalar.activation(y[:], a_sb[bt][:], AF.Sigmoid,
                                         bias=B4[:, bt:bt + 1], scale=scale_ap,
                                         accum_out=s04[:, bt:bt + 1])
                    t2 = yb_pool.tile([128, D_H], bf16, tag="y2")
                    eng = deriv_engine(bt)
                    eng.scalar_tensor_tensor(
                        t2[:], y[:], 1.0, y[:], op0=OP.subtract, op1=OP.mult,
                        accum_out=dneg4[:, bt:bt + 1])
                    if per_tile:
                        s_ = slice(bt, bt + 1)
                        nc.vector.tensor_scalar(dd4[:, s_], dneg4[:, s_],
                                                -DMIN, None, op0=OP.min)
                        nc.vector.reciprocal(rd4[:, s_], dd4[:, s_])
                        nc.vector.scalar_tensor_tensor(
                            u4[:, s_], s04[:, s_], K_TOPK, rd4[:, s_],
                            op0=OP.subtract, op1=OP.mult)
                        nc.vector.tensor_scalar(u4[:, s_], u4[:, s_], CAP,
                                                -CAP, op0=OP.min, op1=OP.max)
                        nc.vector.tensor_tensor(B4[:, s_], B4[:, s_],
                                                u4[:, s_], op=OP.add)
                        if snapshot:
                            nc.vector.tensor_copy(s0p4[:, s_], s04[:, s_])
                            nc.vector.reciprocal(ru4[:, s_], u4[:, s_])
                if not per_tile:
                    nc.vector.tensor_scalar(dd4[:], dneg4[:], -DMIN, None,
                                            op0=OP.min)
                    nc.vector.reciprocal(rd4[:], dd4[:])
                    nc.vector.scalar_tensor_tensor(u4[:], s04[:], K_TOPK,
                                                   rd4[:], op0=OP.subtract,
                                                   op1=OP.mult)
                    nc.vector.tensor_scalar(u4[:], u4[:], CAP, -CAP,
                                            op0=OP.min, op1=OP.max)
                    nc.vector.tensor_tensor(B4[:], B4[:], u4[:], op=OP.add)
                # dependency-chained dummy matmul: keeps the PE HAM clock
                # warm through the solver so the next burst starts fast
                wp = ps_sm.tile([1, 512], f32, tag="warm")
                nc.tensor.matmul(wp[:1, :64], s04[:, t % 4:t % 4 + 1],
                                 a_sb[t % 4][:, :64], start=True, stop=True)

            def pstate_ramp_chain(n):
                """back-to-back junk streams: the PE p-state only reaches full
                clock after ~3us of CONTINUOUS busy (the L1 burst measured
                383ns/500-col matmul vs 622ns for bursts that start cold).
                Queue-ordered after the solver's last dummy, these keep the PE
                continuously busy through the final rounds so the transpose +
                matmul burst runs at full clock."""
                for _ in range(n):
                    wp = ps_sm.tile([1, 512], f32, tag="warm")
                    nc.tensor.matmul(wp[:1, :512], identb[:, 0:1],
                                     xT3[:, 0, :], start=True, stop=True)

            for t in range(PRE_ROUNDS[li]):
                newton_round(c1p_col, t)

            # --- gather result -> global c1 (bcast), lambda = cmaxp/cmax ---
            g8 = sc_pool.tile([1, NCORES], f32, tag=f"g8_{li}")
            nc.sync.dma_start(out=g8[:], in_=cc_out[:])
            M = sc_pool.tile([1, 1], f32, tag=f"M{li}")
            nc.vector.reduce_max(M[:], g8[:], axis=AX.X)
            rc = sc_pool.tile([1, 1], f32, tag=f"rc{li}")
            nc.vector.tensor_tensor(rc[:], M[:], M[:], op=OP.mult)
            nc.vector.tensor_scalar(rc[:], rc[:], 1.0, None, op0=OP.max)
            nc.vector.reciprocal(rc[:], rc[:])
            ps_bg = ps_sm.tile([128, 3], f32, tag="bcast")
            nc.tensor.matmul(ps_bg[:, :1], ones_col[:1, :128], rc[:1, :1],
                             start=True, stop=True)
            cbg = st_pool.tile([128, 2], f32, tag=f"cbg{li}")
            nc.vector.tensor_scalar(cbg[:, 0:1], ps_bg[:, 0:1], -20.0, None,
                                    op0=OP.mult)
            nc.vector.tensor_tensor(cbg[:, 1:2], cmaxp_col, ps_bg[:, 0:1],
                                    op=OP.mult)
            # warm-start rescale: B *= lambda
            nc.vector.scalar_tensor_tensor(B4[:], B4[:], cbg[:, 1:2], zero4[:],
                                           op0=OP.mult, op1=OP.add)

            def secant_round(scale_ap):
                """update round without a derivative pass: d from the secant
                of the previous round's (s0, step).  ACT flows straight into
                the finals (per-tile updates, no 500-wide DVE op)."""
                for bt in range(NBT):
                    y = yb_pool.tile([128, D_H], bf16, tag="yb")
                    nc.scalar.activation(y[:], a_sb[bt][:], AF.Sigmoid,
                                         bias=B4[:, bt:bt + 1], scale=scale_ap,
                                         accum_out=s04[:, bt:bt + 1])
                    s_ = slice(bt, bt + 1)
                    nc.vector.tensor_tensor(dd4[:, s_], s0p4[:, s_],
                                            s04[:, s_], op=OP.subtract)
                    nc.vector.tensor_tensor(dd4[:, s_], dd4[:, s_],
                                            ru4[:, s_], op=OP.mult)
                    nc.vector.tensor_scalar(dd4[:, s_], dd4[:, s_], -DMIN,
                                            None, op0=OP.min)
                    nc.vector.reciprocal(rd4[:, s_], dd4[:, s_])
                    nc.vector.scalar_tensor_tensor(
                        u4[:, s_], s04[:, s_], K_TOPK, rd4[:, s_],
                        op0=OP.subtract, op1=OP.mult)
                    nc.vector.tensor_scalar(u4[:, s_], u4[:, s_], CAP, -CAP,
                                            op0=OP.min, op1=OP.max)
                    nc.vector.tensor_tensor(B4[:, s_], B4[:, s_], u4[:, s_],
                                            op=OP.add)

            if UPD_ROUNDS[li] == 1:
                newton_round(cbg[:, 0:1], 1, per_tile=True)
                pstate_ramp_chain(5)
            else:
                newton_round(cbg[:, 0:1], 1, per_tile=True, snapshot=True)
                # p-state ramp: ~10us of continuous PE busy is needed before
                # the transpose+matmul burst runs at full clock; fill the
                # solver tail without running past the start of the burst
                pstate_ramp_chain(5)
                secant_round(cbg[:, 0:1])
                pstate_ramp_chain(6)

            # --- final fp32 eval + mask apply, pipelined per tile ---
            am_tiles = []
            for bt in range(NBT):
                yf = yf_pool.tile([128, D_H], f32, tag="yf")
                nc.scalar.activation(yf[:], a_sb[bt][:], AF.Sigmoid,
                                     bias=B4[:, bt:bt + 1], scale=cbg[:, 0:1],
                                     accum_out=s04[:, bt:bt + 1])
                rs = st_pool.tile([128, 1], f32, tag=f"rs{bt}")
                nc.vector.reciprocal(rs[:], s04[:, bt:bt + 1])
                rsk = st_pool.tile([128, 1], f32, tag=f"rsk{bt}")
                nc.vector.tensor_scalar(rsk[:], rs[:], K_TOPK, None, op0=OP.mult)
                am = am_pool.tile([128, D_H], bf16, tag="am")
                nc.vector.scalar_tensor_tensor(
                    am[:], yf[:], rsk[:, 0:1], a_sb[bt][:],
                    op0=OP.mult, op1=OP.mult)
                am_tiles.append(am)
            return am_tiles

        def transpose_act(am_tiles):
            """[128,500] x4 batch tiles -> amT [125, KC2, 512] bf16"""
            amT = amt_pool.tile([CH, KC2 * BPC], bf16, tag="amT")
            amT3 = amT[:].rearrange("p (c f) -> p c f", c=KC2)
            for bt in range(NBT):
                p = ps_tr.tile([128, KC2 * 128], bf16, tag="tr")
                p3 = p[:].rearrange("p (c f) -> p c f", c=KC2)
                for nck in range(KC2):
                    nc.tensor.transpose(
                        p3[:CH, nck, :],
                        am_tiles[bt][:, nck * CH:(nck + 1) * CH],
                        identb[:])
                dst = amT3[:, :, bt * 128:(bt + 1) * 128]
                nc.scalar.copy(dst, p3[:CH, :, :])
            return amT3

        # ================= the network =================
        def l1_lhs(kk, bt):
            return xT3[:, kk, bt * 128:(bt + 1) * 128]

        a_ps = mm_layer(l1_lhs, W13, brow[0], D_H, KC1)
        am1 = solve_and_mask(a_ps, 1)
        am1T = transpose_act(am1)

        def l2_lhs(kk, bt):
            return am1T[:, kk, bt * 128:(bt + 1) * 128]

        a_ps = mm_layer(l2_lhs, W23, brow[1], D_H, KC2)
        am2 = solve_and_mask(a_ps, 2)
        am2T = transpose_act(am2)

        def l3_lhs(kk, bt):
            return am2T[:, kk, bt * 128:(bt + 1) * 128]

        a_ps = mm_layer(l3_lhs, W33, brow[2], D_H, KC2)
        am3 = solve_and_mask(a_ps, 3)
        am3T = transpose_act(am3)

        # ---- layer 4, transposed: out' [10, 512] = sum_k W4k^T @ am3T_k ----
        po = ps_mm.tile([128, 512], f32, tag="mm")
        for kk in range(KC2):
            nc.tensor.matmul(po[:D_OUT, :BPC], W43[:, kk, :D_OUT],
                             am3T[:, kk, :],
                             start=(kk == 0),
                             stop=(kk == KC2 - 1) and (brow[3] is None))
        if brow[3] is not None:
            nc.tensor.matmul(po[:D_OUT, :BPC], brow[3][:1, :D_OUT],
                             ones_rowb[:1, :BPC], start=False, stop=True)
        out_sb = singles.tile([D_OUT, BPC], f32, tag="osb")
        nc.vector.tensor_copy(out_sb[:], po[:D_OUT, :BPC])
        nc.sync.dma_start(out=out[:], in_=out_sb[:])

    nc.compile()
    return nc


def _get_nc(masked: bool, zero_bias: bool = False):
    key = (masked, zero_bias)
    if key not in _CACHE:
        _CACHE[key] = _build(masked, zero_bias)
    return _CACHE[key]


def build_in_maps(x, W1, b1, W2, b2, W3, b3, W4, b4):
    import ml_dtypes
    bf = ml_dtypes.bfloat16
    x = np.asarray(x, np.float32)
    common = {
        "W1": np.ascontiguousarray(np.asarray(W1, np.float32).astype(bf)),
        "W2": np.ascontiguousarray(np.asarray(W2, np.float32).astype(bf)),
        "W3": np.ascontiguousarray(np.asarray(W3, np.float32).astype(bf)),
        "W4": np.ascontiguousarray(np.asarray(W4, np.float32).astype(bf)),
        "b1": np.asarray(b1, np.float32).reshape(1, D_H).astype(bf),
        "b2": np.asarray(b2, np.float32).reshape(1, D_H).astype(bf),
        "b3": np.asarray(b3, np.float32).reshape(1, D_H).astype(bf),
        "b4": np.asarray(b4, np.float32).reshape(1, D_OUT).astype(bf),
    }
    in_maps = []
    for c in range(NCORES):
        xs = x[c * BPC:(c + 1) * BPC, :]
        in_maps.append(
            {"xT": np.ascontiguousarray(xs.T.astype(bf)), **common})
    return in_maps


def kernel(x, W1, b1, W2, b2, W3, b3, W4, b4, sparse):
    s = float(np.asarray(sparse))
    assert s in (0.0, 1.0), f"sparse must be 0 or 1, got {s}"
    zb = all(not np.any(np.asarray(b)) for b in (b1, b2, b3, b4))
    nc = _get_nc(masked=(s == 1.0), zero_bias=zb)

    in_maps = build_in_maps(x, W1, b1, W2, b2, W3, b3, W4, b4)
    from concourse.bass_utils import run_bass_kernel_spmd
    res = run_bass_kernel_spmd(nc, in_maps, core_ids=list(range(NCORES)))
    return np.concatenate(
        [np.ascontiguousarray(res.results[c]["out"].T) for c in range(NCORES)],
        axis=0)


if __name__ == "__main__":
    rng = np.random.default_rng(0)
    ins = {
        "x": rng.standard_normal((BS, D_IN), np.float32),
        "W1": rng.standard_normal((D_IN, D_H), np.float32) / np.sqrt(D_IN),
        "b1": np.zeros(D_H, np.float32),
        "W2": rng.standard_normal((D_H, D_H), np.float32) / np.sqrt(D_H),
        "b2": np.zeros(D_H, np.float32),
        "W3": rng.standard_normal((D_H, D_H), np.float32) / np.sqrt(D_H),
        "b3": np.zeros(D_H, np.float32),
        "W4": rng.standard_normal((D_H, D_OUT), np.float32) / np.sqrt(D_H),
        "b4": np.zeros(D_OUT, np.float32),
        "sparse": 1,
    }
    o = kernel(**ins)
    print("out", o.shape, o.dtype, np.abs(o).max())


# revision 23
# speedup vs baseline: 1.1405x; 1.0290x over previous
"""Trainium2 Bass kernel for nn_NeuralNet_62045097558546 (topk_masking).

Network (fp32): 4-layer MLP with SOFT top-k (Sinkhorn) masking after the
first three ReLU layers.  x:[4096,1024] @ W1:[1024,500] -> mask -> @W2[500,500]
-> mask -> @W3[500,500] -> mask -> @W4[500,10].

Math: the reference's 50 Sinkhorn iterations over anchors {0,1} reduce to a
per-row scalar fixed point: solve sum_j sigmoid(c1*a_j + B) = k for the
per-row ACT bias B, where c1 = -20/Cmax and Cmax = max(M^2, 1) with M the
max activation over the FULL batch (one 8-core AllGather of a scalar per
layer).  mask = (k/s0) * sigmoid(c1*a + B).

Schedule (the point of this version): the first collective cannot complete
before ~75us (ncfw entry barrier + first-op setup), and each later AllGather
takes ~6us.  We hide Newton iterations inside those windows by PRESOLVING
with a predicted Cmax_pre = (1.13*local_max)^2 (engines are otherwise idle
while the gather is in flight), then warm-start the global solve with
B *= c1_glob/c1_pre and run 2 guarded Newton update rounds + a final fp32
sigmoid eval that directly yields the mask.  Matmul path (x, W, masked
activations, PE transposes) runs in bf16 (2x PE stream rate, half DMA);
solver input activations stay fp32.  ReLU is done on DVE/GpSimd max-copies
so ACT keeps its sigmoid table loaded (one table load, in the DMA shadow).
Layer 4 is computed transposed (out' = W4^T-chunks x am3T, free dim 512
instead of 10); the host transposes the [10,512] result back.

Validated on CPU sim vs the 50-iteration reference: rel err ~9e-3 sim-units
(sim overestimates: the same sim scores the previous 197us kernel's schedule
at ~4e-3 while it measures 7e-4 on HW).
"""

import numpy as np
from contextlib import ExitStack

BS, D_IN, D_H, D_OUT = 4096, 1024, 500, 10
NCORES = 8
BPC = BS // NCORES            # 512 batch rows per core
NBT = BPC // 128              # 4 batch tiles of 128
KC1 = D_IN // 128             # 8 contraction chunks for layer 1
CH = 125                      # contraction chunk size for 500-dim layers
KC2 = D_H // CH               # 4 chunks
K_TOPK = 400.0
PRED2_SHARD = 1.28            # (global/shard-max)^2 predictor
PRE_ROUNDS = [3, 2, 2]        # local-Cmax presolve rounds per layer
UPD_ROUNDS = [1, 1, 1]        # global-Cmax Newton update rounds per layer
DMIN = 2.0                    # |d| floor (negated-d convention)
CAP = 8.0                     # Newton step clamp

_CACHE = {}


def _build(masked: bool, zero_bias: bool = False):
    import concourse.bass as bass
    import concourse.bacc as bacc
    import concourse.mybir as mybir
    import concourse.tile as tile
    from concourse import masks as cmasks

    f32 = mybir.dt.float32
    bf16 = mybir.dt.bfloat16
    AX = mybir.AxisListType
    OP = mybir.AluOpType
    AF = mybir.ActivationFunctionType

    nc = bacc.Bacc("TRN2", target_bir_lowering=False, debug=False,
                   num_devices=NCORES)

    xT = nc.dram_tensor("xT", [D_IN, BPC], bf16, kind="ExternalInput")
    W1 = nc.dram_tensor("W1", [D_IN, D_H], bf16, kind="ExternalInput")
    W2 = nc.dram_tensor("W2", [D_H, D_H], bf16, kind="ExternalInput")
    W3 = nc.dram_tensor("W3", [D_H, D_H], bf16, kind="ExternalInput")
    W4 = nc.dram_tensor("W4", [D_H, D_OUT], bf16, kind="ExternalInput")
    b1 = nc.dram_tensor("b1", [1, D_H], bf16, kind="ExternalInput")
    b2 = nc.dram_tensor("b2", [1, D_H], bf16, kind="ExternalInput")
    b3 = nc.dram_tensor("b3", [1, D_H], bf16, kind="ExternalInput")
    b4 = nc.dram_tensor("b4", [1, D_OUT], bf16, kind="ExternalInput")
    # transposed output [D_OUT, BPC]; the host transposes back
    out = nc.dram_tensor("out", [D_OUT, BPC], f32, kind="ExternalOutput")

    with tile.TileContext(nc) as tc, ExitStack() as ctx:
        singles = ctx.enter_context(tc.tile_pool(name="singles", bufs=1))
        a_pool = ctx.enter_context(tc.tile_pool(name="a", bufs=NBT + 1))
        yb_pool = ctx.enter_context(tc.tile_pool(name="yb", bufs=3))
        yf_pool = ctx.enter_context(tc.tile_pool(name="yf", bufs=3))
        am_pool = ctx.enter_context(tc.tile_pool(name="am", bufs=NBT))
        amt_pool = ctx.enter_context(tc.tile_pool(name="amt", bufs=2))
        st_pool = ctx.enter_context(tc.tile_pool(name="st", bufs=24))
        sc_pool = ctx.enter_context(tc.tile_pool(name="sc", bufs=2))
        ps_mm = ctx.enter_context(tc.tile_pool(name="ps_mm", bufs=3, space="PSUM"))
        ps_tr = ctx.enter_context(tc.tile_pool(name="ps_tr", bufs=2, space="PSUM"))
        ps_sm = ctx.enter_context(tc.tile_pool(name="ps_sm", bufs=1, space="PSUM"))
        dram = ctx.enter_context(tc.tile_pool(name="dram", bufs=8, space="DRAM"))

        # ---- constants ----
        identf = singles.tile([128, 128], f32, tag="identf")
        cmasks.make_identity(nc, identf[:])
        identb = singles.tile([128, 128], bf16, tag="identb")
        nc.vector.tensor_copy(identb[:], identf[:])
        ones_col = singles.tile([1, 128], f32, tag="ones")
        nc.vector.memset(ones_col[:], 1.0)
        zero4 = singles.tile([128, 4], f32, tag="zero4")
        nc.vector.memset(zero4[:], 0.0)
        junk1 = singles.tile([128, 1], f32, tag="junk1")
        nc.vector.memset(junk1[:], 0.0)

        # ---- weight / input loads (HWDGE), split over two rings ----
        xT_sb = singles.tile([128, KC1 * BPC], bf16, tag="xT")
        xT3 = xT_sb[:].rearrange("p (c f) -> p c f", c=KC1)
        xTd = xT[:].rearrange("(c p) f -> p c f", p=128)
        W1_sb = singles.tile([128, KC1 * D_H], bf16, tag="W1")
        W13 = W1_sb[:].rearrange("p (c f) -> p c f", c=KC1)
        W1d = W1[:].rearrange("(c p) f -> p c f", p=128)
        for kk in range(KC1):
            nc.sync.dma_start(out=xT3[:, kk, :], in_=xTd[:, kk, :])
            nc.scalar.dma_start(out=W13[:, kk, :], in_=W1d[:, kk, :])

        W2_sb = singles.tile([CH, KC2 * D_H], bf16, tag="W2")
        W23 = W2_sb[:].rearrange("p (c f) -> p c f", c=KC2)
        nc.sync.dma_start(out=W23, in_=W2[:].rearrange("(c p) f -> p c f", p=CH))

        W3_sb = singles.tile([CH, KC2 * D_H], bf16, tag="W3")
        W33 = W3_sb[:].rearrange("p (c f) -> p c f", c=KC2)
        nc.scalar.dma_start(out=W33, in_=W3[:].rearrange("(c p) f -> p c f", p=CH))

        W4_sb = singles.tile([CH, KC2 * D_OUT], bf16, tag="W4")
        W43 = W4_sb[:].rearrange("p (c f) -> p c f", c=KC2)
        nc.sync.dma_start(out=W43, in_=W4[:].rearrange("(c p) f -> p c f", p=CH))

        brow = [None] * 4
        ones_rowb = None
        if not zero_bias:
            for i, bt_dram in enumerate([b1, b2, b3, b4]):
                n = D_OUT if i == 3 else D_H
                t = singles.tile([1, n], bf16, tag=f"b{i+1}", name=f"brow{i+1}")
                nc.scalar.dma_start(out=t[:], in_=bt_dram[:])
                brow[i] = t
            ones_colb = singles.tile([1, 128], bf16, tag="onesb")
            nc.vector.tensor_copy(ones_colb[:], ones_col[:])
            ones_rowb = singles.tile([1, BPC], bf16, tag="onesrow")
            nc.vector.memset(ones_rowb[:], 1.0)

        if masked:
            # preload the sigmoid spline table while DMAs stream (the first
            # real sigmoid otherwise pays ~1.3us mid-phase)
            dummy_y = sc_pool.tile([128, 1], bf16, tag="dummy_y")
            nc.scalar.activation(dummy_y[:], junk1[:], AF.Sigmoid)

        def mm_layer(lhs_chunks, w3d, brow_t, nfree, kc):
            """emit matmuls; returns psum tiles [128,512] per batch tile"""
            ps = []
            for bt in range(NBT):
                p = ps_mm.tile([128, 512], f32, tag="mm")
                for kk in range(kc):
                    last = (kk == kc - 1) and (brow_t is None)
                    nc.tensor.matmul(
                        p[:, :nfree],
                        lhs_chunks(kk, bt),
                        w3d[:, kk, :nfree],
                        start=(kk == 0), stop=last)
                if brow_t is not None:
                    nc.tensor.matmul(p[:, :nfree],
                                     ones_colb[:1, :128],
                                     brow_t[:1, :nfree],
                                     start=False, stop=True)
                ps.append(p)
            return ps

        def solve_and_mask(a_ps, layer):
            """a_ps: psum tiles [128,512](:D_H) pre-relu.  Returns am tiles
            [128, D_H] bf16 in SBUF (masked activations)."""
            if not masked:
                am_tiles = []
                for bt in range(NBT):
                    am = am_pool.tile([128, D_H], bf16, tag="am")
                    nc.vector.tensor_scalar(am[:], a_ps[bt][:, :D_H], 0.0,
                                            None, op0=OP.max)
                    am_tiles.append(am)
                return am_tiles

            li = layer - 1
            # --- local rowmax -> gather trigger, ASAP ---
            rm4 = st_pool.tile([128, 4], f32, tag=f"rm4_{li}", name=f"rm4_{li}")
            for bt in range(NBT):
                nc.vector.reduce_max(rm4[:, bt:bt + 1], a_ps[bt][:, :D_H],
                                     axis=AX.X)
            mall = st_pool.tile([128, 1], f32, tag=f"mall{li}")
            nc.vector.reduce_max(mall[:], rm4[:], axis=AX.X)
            nc.vector.tensor_scalar(mall[:], mall[:], 0.0, None, op0=OP.max)
            # scalar AllGather (a [128,1]-strided cc_in DMA pays ~90ns/element
            # descriptor overhead -> 12us; one 4B element is fast)
            pst = ps_sm.tile([1, 128], f32, tag="pmax")
            nc.tensor.transpose(pst[:1, :128], mall[:, :1], identf[:])
            locmax = sc_pool.tile([1, 1], f32, tag=f"locmax{li}",
                                  name=f"locmax{li}")
            nc.vector.reduce_max(locmax[:], pst[:1, :128], axis=AX.X)
            cc_in = dram.tile([1, 1], f32, tag="ccin")
            cc_out = dram.tile([1, NCORES], f32, tag="ccout")
            nc.gpsimd.dma_start(out=cc_in[:], in_=locmax[:])
            nc.gpsimd.collective_compute(
                "AllGather", OP.bypass,
                replica_groups=[list(range(NCORES))],
                ins=[cc_in[:]], outs=[cc_out[:]])

            # --- predicted-Cmax (runs during the gather) ---
            # cmaxp = max(1, PRED2*M_loc^2); c1p = -20/cmaxp; c2p = 10/cmaxp
            # shard-scalar predictor broadcast via PE rank-1 (the per-partition
            # predictor is noisier and needs a second update round -- with the
            # shard predictor one post-gather Newton round suffices)
            c1p3 = sc_pool.tile([1, 3], f32, tag="c1p3", name=f"c1p3_{li}")
            nc.vector.scalar_tensor_tensor(
                c1p3[:, 2:3], locmax[:], PRED2_SHARD, locmax[:],
                op0=OP.mult, op1=OP.mult)
            nc.vector.tensor_scalar(c1p3[:, 2:3], c1p3[:, 2:3], 1.0, None,
                                    op0=OP.max)
            rcp = sc_pool.tile([1, 1], f32, tag="rcp")
            nc.vector.reciprocal(rcp[:], c1p3[:, 2:3])
            nc.vector.tensor_scalar(c1p3[:, 0:1], rcp[:], -20.0, None,
                                    op0=OP.mult)
            nc.vector.tensor_scalar(c1p3[:, 1:2], rcp[:], 10.0, None,
                                    op0=OP.mult)
            ps_bp = ps_sm.tile([128, 3], f32, tag="bcast")
            nc.tensor.matmul(ps_bp[:, :3], ones_col[:1, :128],
                             c1p3[:1, :3], start=True, stop=True)
            cbp = st_pool.tile([128, 3], f32, tag=f"cbp{li}")
            nc.vector.tensor_copy(cbp[:], ps_bp[:, :3])
            c1p_col, c2p_col, cmaxp_col = (cbp[:, 0:1], cbp[:, 1:2],
                                           cbp[:, 2:3])

            # --- packed per-row solver state ---
            B4 = st_pool.tile([128, 4], f32, tag=f"B4_{li}", name=f"B4_{li}")
            for bt in range(NBT):
                nc.vector.tensor_copy(B4[:, bt:bt + 1], c2p_col)
            s04 = st_pool.tile([128, 4], f32, tag=f"s04_{li}", name=f"s04_{li}")
            dneg4 = st_pool.tile([128, 4], f32, tag=f"dn4_{li}")
            dd4 = st_pool.tile([128, 4], f32, tag=f"dd4_{li}")
            rd4 = st_pool.tile([128, 4], f32, tag=f"rd4_{li}")
            u4 = st_pool.tile([128, 4], f32, tag=f"u4_{li}")

            # --- relu copies (fp32 solver input): tiles 0/1 on ACT (Relu is
            # in the same table set as Sigmoid), 2/3 on DVE ---
            a_sb = []
            for bt in range(NBT):
                a = a_pool.tile([128, D_H], f32, tag="a")
                if bt < 3:
                    nc.scalar.activation(a[:], a_ps[bt][:, :D_H], AF.Relu)
                else:
                    nc.vector.tensor_scalar(a[:], a_ps[bt][:, :D_H], 0.0,
                                            None, op0=OP.max)
                a_sb.append(a)

            def deriv_engine(bt):
                # (gpsimd rejects scalar_tensor_tensor at BIR verify; keep DVE)
                return nc.vector

            s0p4 = st_pool.tile([128, 4], f32, tag=f"s0p4_{li}")
            ru4 = st_pool.tile([128, 4], f32, tag=f"ru4_{li}")

            def newton_round(scale_ap, t, per_tile=False, snapshot=False):
                """one Newton round; per_tile=True updates B per batch tile so
                the next ACT pass can start without a barrier.  snapshot saves
                (s0, 1/u) per tile for a following secant round."""
                for bt in range(NBT):
                    y = yb_pool.tile([128, D_H], bf16, tag="yb")
                    nc.scalar.activation(y[:], a_sb[bt][:], AF.Sigmoid,
                                         bias=B4[:, bt:bt + 1], scale=scale_ap,
                                         accum_out=s04[:, bt:bt + 1])
                    t2 = yb_pool.tile([128, D_H], bf16, tag="y2")
                    eng = deriv_engine(bt)
                    eng.scalar_tensor_tensor(
                        t2[:], y[:], 1.0, y[:], op0=OP.subtract, op1=OP.mult,
                        accum_out=dneg4[:, bt:bt + 1])
                    if per_tile:
                        s_ = slice(bt, bt + 1)
                        nc.vector.tensor_scalar(dd4[:, s_], dneg4[:, s_],
                                                -DMIN, None, op0=OP.min)
                        nc.vector.reciprocal(rd4[:, s_], dd4[:, s_])
                        nc.vector.scalar_tensor_tensor(
                            u4[:, s_], s04[:, s_], K_TOPK, rd4[:, s_],
                            op0=OP.subtract, op1=OP.mult)
                        nc.vector.tensor_scalar(u4[:, s_], u4[:, s_], CAP,
                                                -CAP, op0=OP.min, op1=OP.max)
                        nc.vector.tensor_tensor(B4[:, s_], B4[:, s_],
                                                u4[:, s_], op=OP.add)
                        if snapshot:
                            nc.vector.tensor_copy(s0p4[:, s_], s04[:, s_])
                            nc.vector.reciprocal(ru4[:, s_], u4[:, s_])
                if not per_tile:
                    nc.vector.tensor_scalar(dd4[:], dneg4[:], -DMIN, None,
                                            op0=OP.min)
                    nc.vector.reciprocal(rd4[:], dd4[:])
                    nc.vector.scalar_tensor_tensor(u4[:], s04[:], K_TOPK,
                                                   rd4[:], op0=OP.subtract,
                                                   op1=OP.mult)
                    nc.vector.tensor_scalar(u4[:], u4[:], CAP, -CAP,
                                            op0=OP.min, op1=OP.max)
                    nc.vector.tensor_tensor(B4[:], B4[:], u4[:], op=OP.add)
                # dependency-chained dummy matmul: keeps the PE HAM clock
                # warm through the solver so the next burst starts fast
                wp = ps_sm.tile([1, 512], f32, tag="warm")
                nc.tensor.matmul(wp[:1, :64], s04[:, t % 4:t % 4 + 1],
                                 a_sb[t % 4][:, :64], start=True, stop=True)

            def pstate_ramp_chain(n):
                """back-to-back junk streams: the PE p-state only reaches full
                clock after ~3us of CONTINUOUS busy (the L1 burst measured
                383ns/500-col matmul vs 622ns for bursts that start cold).
                Queue-ordered after the solver's last dummy, these keep the PE
                continuously busy through the final rounds so the transpose +
                matmul burst runs at full clock."""
                for _ in range(n):
                    wp = ps_sm.tile([1, 512], f32, tag="warm")
                    nc.tensor.matmul(wp[:1, :512], identb[:, 0:1],
                                     xT3[:, 0, :], start=True, stop=True)

            for t in range(PRE_ROUNDS[li]):
                newton_round(c1p_col, t)

            # --- gather result -> global c1 (bcast), lambda = cmaxp/cmax ---
            g8 = sc_pool.tile([1, NCORES], f32, tag=f"g8_{li}")
            nc.sync.dma_start(out=g8[:], in_=cc_out[:])
            M = sc_pool.tile([1, 1], f32, tag=f"M{li}")
            nc.vector.reduce_max(M[:], g8[:], axis=AX.X)
            rc = sc_pool.tile([1, 1], f32, tag=f"rc{li}")
            nc.vector.tensor_tensor(rc[:], M[:], M[:], op=OP.mult)
            nc.vector.tensor_scalar(rc[:], rc[:], 1.0, None, op0=OP.max)
            nc.vector.reciprocal(rc[:], rc[:])
            ps_bg = ps_sm.tile([128, 3], f32, tag="bcast")
            nc.tensor.matmul(ps_bg[:, :1], ones_col[:1, :128], rc[:1, :1],
                             start=True, stop=True)
            cbg = st_pool.tile([128, 2], f32, tag=f"cbg{li}")
            nc.vector.tensor_scalar(cbg[:, 0:1], ps_bg[:, 0:1], -20.0, None,
                                    op0=OP.mult)
            nc.vector.tensor_tensor(cbg[:, 1:2], cmaxp_col, ps_bg[:, 0:1],
                                    op=OP.mult)
            # warm-start rescale: B *= lambda
            nc.vector.scalar_tensor_tensor(B4[:], B4[:], cbg[:, 1:2], zero4[:],
                                           op0=OP.mult, op1=OP.add)

            def secant_round(scale_ap):
                """update round without a derivative pass: d from the secant
                of the previous round's (s0, step).  ACT flows straight into
                the finals (per-tile updates, no 500-wide DVE op)."""
                for bt in range(NBT):
                    y = yb_pool.tile([128, D_H], bf16, tag="yb")
                    nc.scalar.activation(y[:], a_sb[bt][:], AF.Sigmoid,
                                         bias=B4[:, bt:bt + 1], scale=scale_ap,
                                         accum_out=s04[:, bt:bt + 1])
                    s_ = slice(bt, bt + 1)
                    nc.vector.tensor_tensor(dd4[:, s_], s0p4[:, s_],
                                            s04[:, s_], op=OP.subtract)
                    nc.vector.tensor_tensor(dd4[:, s_], dd4[:, s_],
                                            ru4[:, s_], op=OP.mult)
                    nc.vector.tensor_scalar(dd4[:, s_], dd4[:, s_], -DMIN,
                                            None, op0=OP.min)
                    nc.vector.reciprocal(rd4[:, s_], dd4[:, s_])
                    nc.vector.scalar_tensor_tensor(
                        u4[:, s_], s04[:, s_], K_TOPK, rd4[:, s_],
                        op0=OP.subtract, op1=OP.mult)
                    nc.vector.tensor_scalar(u4[:, s_], u4[:, s_], CAP, -CAP,
                                            op0=OP.min, op1=OP.max)
                    nc.vector.tensor_tensor(B4[:, s_], B4[:, s_], u4[:, s_],
                                            op=OP.add)

            if UPD_ROUNDS[li] == 1:
                newton_round(cbg[:, 0:1], 1, per_tile=True)
                pstate_ramp_chain(5)
            else:
                newton_round(cbg[:, 0:1], 1, per_tile=True, snapshot=True)
                # p-state ramp: ~10us of continuous PE busy is needed before
                # the transpose+matmul burst runs at full clock; fill the
                # solver tail without running past the start of the burst
                pstate_ramp_chain(5)
                secant_round(cbg[:, 0:1])
                pstate_ramp_chain(6)

            # --- final fp32 eval + mask apply, pipelined per tile ---
            am_tiles = []
            for bt in range(NBT):
                yf = yf_pool.tile([128, D_H], f32, tag="yf")
                nc.scalar.activation(yf[:], a_sb[bt][:], AF.Sigmoid,
                                     bias=B4[:, bt:bt + 1], scale=cbg[:, 0:1],
                                     accum_out=s04[:, bt:bt + 1])
                rs = st_pool.tile([128, 1], f32, tag=f"rs{bt}")
                nc.vector.reciprocal(rs[:], s04[:, bt:bt + 1])
                rsk = st_pool.tile([128, 1], f32, tag=f"rsk{bt}")
                nc.vector.tensor_scalar(rsk[:], rs[:], K_TOPK, None, op0=OP.mult)
                am = am_pool.tile([128, D_H], bf16, tag="am")
                nc.vector.scalar_tensor_tensor(
                    am[:], yf[:], rsk[:, 0:1], a_sb[bt][:],
                    op0=OP.mult, op1=OP.mult)
                am_tiles.append(am)
            return am_tiles

        def transpose_act(am_tiles):
            """[128,500] x4 batch tiles -> amT [125, KC2, 512] bf16"""
            amT = amt_pool.tile([CH, KC2 * BPC], bf16, tag="amT")
            amT3 = amT[:].rearrange("p (c f) -> p c f", c=KC2)
            for bt in range(NBT):
                p = ps_tr.tile([128, KC2 * 128], bf16, tag="tr")
                p3 = p[:].rearrange("p (c f) -> p c f", c=KC2)
                for nck in range(KC2):
                    nc.tensor.transpose(
                        p3[:CH, nck, :],
                        am_tiles[bt][:, nck * CH:(nck + 1) * CH],
                        identb[:])
                dst = amT3[:, :, bt * 128:(bt + 1) * 128]
                nc.scalar.copy(dst, p3[:CH, :, :])
            return amT3

        # ================= the network =================
        def l1_lhs(kk, bt):
            return xT3[:, kk, bt * 128:(bt + 1) * 128]

        a_ps = mm_layer(l1_lhs, W13, brow[0], D_H, KC1)
        am1 = solve_and_mask(a_ps, 1)
        am1T = transpose_act(am1)

        def l2_lhs(kk, bt):
            return am1T[:, kk, bt * 128:(bt + 1) * 128]

        a_ps = mm_layer(l2_lhs, W23, brow[1], D_H, KC2)
        am2 = solve_and_mask(a_ps, 2)
        am2T = transpose_act(am2)

        def l3_lhs(kk, bt):
            return am2T[:, kk, bt * 128:(bt + 1) * 128]

        a_ps = mm_layer(l3_lhs, W33, brow[2], D_H, KC2)
        am3 = solve_and_mask(a_ps, 3)
        am3T = transpose_act(am3)

        # ---- layer 4, transposed: out' [10, 512] = sum_k W4k^T @ am3T_k ----
        po = ps_mm.tile([128, 512], f32, tag="mm")
        for kk in range(KC2):
            nc.tensor.matmul(po[:D_OUT, :BPC], W43[:, kk, :D_OUT],
                             am3T[:, kk, :],
                             start=(kk == 0),
                             stop=(kk == KC2 - 1) and (brow[3] is None))
        if brow[3] is not None:
            nc.tensor.matmul(po[:D_OUT, :BPC], brow[3][:1, :D_OUT],
                             ones_rowb[:1, :BPC], start=False, stop=True)
        out_sb = singles.tile([D_OUT, BPC], f32, tag="osb")
        nc.vector.tensor_copy(out_sb[:], po[:D_OUT, :BPC])
        nc.sync.dma_start(out=out[:], in_=out_sb[:])

    nc.compile()
    return nc


def _get_nc(masked: bool, zero_bias: bool = False):
    key = (masked, zero_bias)
    if key not in _CACHE:
        _CACHE[key] = _build(masked, zero_bias)
    return _CACHE[key]


def build_in_maps(x, W1, b1, W2, b2, W3, b3, W4, b4):
    import ml_dtypes
    bf = ml_dtypes.bfloat16
    x = np.asarray(x, np.float32)
    common = {
        "W1": np.ascontiguousarray(np.asarray(W1, np.float32).astype(bf)),
        "W2": np.ascontiguousarray(np.asarray(W2, np.float32).astype(bf)),
        "W3": np.ascontiguousarray(np.asarray(W3, np.float32).astype(bf)),
        "W4": np.ascontiguousarray(np.asarray(W4, np.float32).astype(bf)),
        "b1": np.asarray(b1, np.float32).reshape(1, D_H).astype(bf),
        "b2": np.asarray(b2, np.float32).reshape(1, D_H).astype(bf),
        "b3": np.asarray(b3, np.float32).reshape(1, D_H).astype(bf),
        "b4": np.asarray(b4, np.float32).reshape(1, D_OUT).astype(bf),
    }
    in_maps = []
    for c in range(NCORES):
        xs = x[c * BPC:(c + 1) * BPC, :]
        in_maps.append(
            {"xT": np.ascontiguousarray(xs.T.astype(bf)), **common})
    return in_maps


def kernel(x, W1, b1, W2, b2, W3, b3, W4, b4, sparse):
    s = float(np.asarray(sparse))
    assert s in (0.0, 1.0), f"sparse must be 0 or 1, got {s}"
    zb = all(not np.any(np.asarray(b)) for b in (b1, b2, b3, b4))
    nc = _get_nc(masked=(s == 1.0), zero_bias=zb)

    in_maps = build_in_maps(x, W1, b1, W2, b2, W3, b3, W4, b4)
    from concourse.bass_utils import run_bass_kernel_spmd
    res = run_bass_kernel_spmd(nc, in_maps, core_ids=list(range(NCORES)))
    return np.concatenate(
        [np.ascontiguousarray(res.results[c]["out"].T) for c in range(NCORES)],
        axis=0)


if __name__ == "__main__":
    rng = np.random.default_rng(0)
    ins = {
        "x": rng.standard_normal((BS, D_IN), np.float32),
        "W1": rng.standard_normal((D_IN, D_H), np.float32) / np.sqrt(D_IN),
        "b1": np.zeros(D_H, np.float32),
        "W2": rng.standard_normal((D_H, D_H), np.float32) / np.sqrt(D_H),
        "b2": np.zeros(D_H, np.float32),
        "W3": rng.standard_normal((D_H, D_H), np.float32) / np.sqrt(D_H),
        "b3": np.zeros(D_H, np.float32),
        "W4": rng.standard_normal((D_H, D_OUT), np.float32) / np.sqrt(D_H),
        "b4": np.zeros(D_OUT, np.float32),
        "sparse": 1,
    }
    o = kernel(**ins)
    print("out", o.shape, o.dtype, np.abs(o).max())


# revision 25
# speedup vs baseline: 1.2120x; 1.0626x over previous
"""Trainium2 Bass kernel for nn_NeuralNet_62045097558546 (topk_masking).

Network (fp32): 4-layer MLP with SOFT top-k (Sinkhorn) masking after the
first three ReLU layers.  x:[4096,1024] @ W1:[1024,500] -> mask -> @W2[500,500]
-> mask -> @W3[500,500] -> mask -> @W4[500,10].

Math: the reference's 50 Sinkhorn iterations over anchors {0,1} reduce to a
per-row scalar fixed point: solve sum_j sigmoid(c1*a_j + B) = k for the
per-row ACT bias B, where c1 = -20/Cmax and Cmax = max(M^2, 1) with M the
max activation over the FULL batch (one 8-core AllGather of a scalar per
layer).  mask = (k/s0) * sigmoid(c1*a + B).

Schedule (the point of this version): the first collective cannot complete
before ~75us (ncfw entry barrier + first-op setup), and each later AllGather
takes ~6us.  We hide Newton iterations inside those windows by PRESOLVING
with a predicted Cmax_pre = (1.13*local_max)^2 (engines are otherwise idle
while the gather is in flight), then warm-start the global solve with
B *= c1_glob/c1_pre and run 2 guarded Newton update rounds + a final fp32
sigmoid eval that directly yields the mask.  Matmul path (x, W, masked
activations, PE transposes) runs in bf16 (2x PE stream rate, half DMA);
solver input activations stay fp32.  ReLU is done on DVE/GpSimd max-copies
so ACT keeps its sigmoid table loaded (one table load, in the DMA shadow).
Layer 4 is computed transposed (out' = W4^T-chunks x am3T, free dim 512
instead of 10); the host transposes the [10,512] result back.

Validated on CPU sim vs the 50-iteration reference: rel err ~9e-3 sim-units
(sim overestimates: the same sim scores the previous 197us kernel's schedule
at ~4e-3 while it measures 7e-4 on HW).
"""

import numpy as np
from contextlib import ExitStack

BS, D_IN, D_H, D_OUT = 4096, 1024, 500, 10
NCORES = 8
BPC = BS // NCORES            # 512 batch rows per core
NBT = BPC // 128              # 4 batch tiles of 128
KC1 = D_IN // 128             # 8 contraction chunks for layer 1
CH = 125                      # contraction chunk size for 500-dim layers
KC2 = D_H // CH               # 4 chunks
K_TOPK = 400.0
PRED2_SHARD = 1.28            # (global/shard-max)^2 predictor
PRE_ROUNDS = [3, 1, 2]        # local-Cmax presolve rounds per layer
UPD_ROUNDS = [1, 1, 1]        # global-Cmax Newton update rounds per layer
DMIN = 2.0                    # |d| floor (negated-d convention)
CAP = 8.0                     # Newton step clamp

_CACHE = {}


def _build(masked: bool, zero_bias: bool = False):
    import concourse.bass as bass
    import concourse.bacc as bacc
    import concourse.mybir as mybir
    import concourse.tile as tile
    from concourse import masks as cmasks

    f32 = mybir.dt.float32
    bf16 = mybir.dt.bfloat16
    AX = mybir.AxisListType
    OP = mybir.AluOpType
    AF = mybir.ActivationFunctionType

    nc = bacc.Bacc("TRN2", target_bir_lowering=False, debug=False,
                   num_devices=NCORES)

    xT = nc.dram_tensor("xT", [D_IN, BPC], bf16, kind="ExternalInput")
    W1 = nc.dram_tensor("W1", [D_IN, D_H], bf16, kind="ExternalInput")
    W2 = nc.dram_tensor("W2", [D_H, D_H], bf16, kind="ExternalInput")
    W3 = nc.dram_tensor("W3", [D_H, D_H], bf16, kind="ExternalInput")
    W4 = nc.dram_tensor("W4", [D_H, D_OUT], bf16, kind="ExternalInput")
    b1 = nc.dram_tensor("b1", [1, D_H], bf16, kind="ExternalInput")
    b2 = nc.dram_tensor("b2", [1, D_H], bf16, kind="ExternalInput")
    b3 = nc.dram_tensor("b3", [1, D_H], bf16, kind="ExternalInput")
    b4 = nc.dram_tensor("b4", [1, D_OUT], bf16, kind="ExternalInput")
    # transposed output [D_OUT, BPC]; the host transposes back
    out = nc.dram_tensor("out", [D_OUT, BPC], f32, kind="ExternalOutput")

    with tile.TileContext(nc) as tc, ExitStack() as ctx:
        singles = ctx.enter_context(tc.tile_pool(name="singles", bufs=1))
        a_pool = ctx.enter_context(tc.tile_pool(name="a", bufs=NBT + 1))
        yb_pool = ctx.enter_context(tc.tile_pool(name="yb", bufs=3))
        yf_pool = ctx.enter_context(tc.tile_pool(name="yf", bufs=3))
        am_pool = ctx.enter_context(tc.tile_pool(name="am", bufs=NBT))
        amt_pool = ctx.enter_context(tc.tile_pool(name="amt", bufs=2))
        st_pool = ctx.enter_context(tc.tile_pool(name="st", bufs=24))
        sc_pool = ctx.enter_context(tc.tile_pool(name="sc", bufs=2))
        ps_mm = ctx.enter_context(tc.tile_pool(name="ps_mm", bufs=3, space="PSUM"))
        ps_tr = ctx.enter_context(tc.tile_pool(name="ps_tr", bufs=2, space="PSUM"))
        ps_sm = ctx.enter_context(tc.tile_pool(name="ps_sm", bufs=1, space="PSUM"))
        dram = ctx.enter_context(tc.tile_pool(name="dram", bufs=8, space="DRAM"))

        # ---- constants ----
        identf = singles.tile([128, 128], f32, tag="identf")
        cmasks.make_identity(nc, identf[:])
        identb = singles.tile([128, 128], bf16, tag="identb")
        nc.vector.tensor_copy(identb[:], identf[:])
        ones_col = singles.tile([1, 128], f32, tag="ones")
        nc.vector.memset(ones_col[:], 1.0)
        zero4 = singles.tile([128, 4], f32, tag="zero4")
        nc.vector.memset(zero4[:], 0.0)
        junk1 = singles.tile([128, 1], f32, tag="junk1")
        nc.vector.memset(junk1[:], 0.0)

        # ---- weight / input loads (HWDGE), split over two rings ----
        xT_sb = singles.tile([128, KC1 * BPC], bf16, tag="xT")
        xT3 = xT_sb[:].rearrange("p (c f) -> p c f", c=KC1)
        xTd = xT[:].rearrange("(c p) f -> p c f", p=128)
        W1_sb = singles.tile([128, KC1 * D_H], bf16, tag="W1")
        W13 = W1_sb[:].rearrange("p (c f) -> p c f", c=KC1)
        W1d = W1[:].rearrange("(c p) f -> p c f", p=128)
        for kk in range(KC1):
            nc.sync.dma_start(out=xT3[:, kk, :], in_=xTd[:, kk, :])
            nc.scalar.dma_start(out=W13[:, kk, :], in_=W1d[:, kk, :])

        W2_sb = singles.tile([CH, KC2 * D_H], bf16, tag="W2")
        W23 = W2_sb[:].rearrange("p (c f) -> p c f", c=KC2)
        nc.sync.dma_start(out=W23, in_=W2[:].rearrange("(c p) f -> p c f", p=CH))

        W3_sb = singles.tile([CH, KC2 * D_H], bf16, tag="W3")
        W33 = W3_sb[:].rearrange("p (c f) -> p c f", c=KC2)
        nc.scalar.dma_start(out=W33, in_=W3[:].rearrange("(c p) f -> p c f", p=CH))

        W4_sb = singles.tile([CH, KC2 * D_OUT], bf16, tag="W4")
        W43 = W4_sb[:].rearrange("p (c f) -> p c f", c=KC2)
        nc.sync.dma_start(out=W43, in_=W4[:].rearrange("(c p) f -> p c f", p=CH))

        brow = [None] * 4
        ones_rowb = None
        if not zero_bias:
            for i, bt_dram in enumerate([b1, b2, b3, b4]):
                n = D_OUT if i == 3 else D_H
                t = singles.tile([1, n], bf16, tag=f"b{i+1}", name=f"brow{i+1}")
                nc.scalar.dma_start(out=t[:], in_=bt_dram[:])
                brow[i] = t
            ones_colb = singles.tile([1, 128], bf16, tag="onesb")
            nc.vector.tensor_copy(ones_colb[:], ones_col[:])
            ones_rowb = singles.tile([1, BPC], bf16, tag="onesrow")
            nc.vector.memset(ones_rowb[:], 1.0)

        if masked:
            # preload the sigmoid spline table while DMAs stream (the first
            # real sigmoid otherwise pays ~1.3us mid-phase)
            dummy_y = sc_pool.tile([128, 1], bf16, tag="dummy_y")
            nc.scalar.activation(dummy_y[:], junk1[:], AF.Sigmoid)

        def mm_layer(lhs_chunks, w3d, brow_t, nfree, kc):
            """emit matmuls; returns psum tiles [128,512] per batch tile"""
            ps = []
            for bt in range(NBT):
                p = ps_mm.tile([128, 512], f32, tag="mm")
                for kk in range(kc):
                    last = (kk == kc - 1) and (brow_t is None)
                    nc.tensor.matmul(
                        p[:, :nfree],
                        lhs_chunks(kk, bt),
                        w3d[:, kk, :nfree],
                        start=(kk == 0), stop=last)
                if brow_t is not None:
                    nc.tensor.matmul(p[:, :nfree],
                                     ones_colb[:1, :128],
                                     brow_t[:1, :nfree],
                                     start=False, stop=True)
                ps.append(p)
            return ps

        def solve_and_mask(a_ps, layer):
            """a_ps: psum tiles [128,512](:D_H) pre-relu.  Returns am tiles
            [128, D_H] bf16 in SBUF (masked activations)."""
            if not masked:
                am_tiles = []
                for bt in range(NBT):
                    am = am_pool.tile([128, D_H], bf16, tag="am")
                    nc.vector.tensor_scalar(am[:], a_ps[bt][:, :D_H], 0.0,
                                            None, op0=OP.max)
                    am_tiles.append(am)
                return am_tiles

            li = layer - 1
            # --- local rowmax -> gather trigger, ASAP ---
            rm4 = st_pool.tile([128, 4], f32, tag=f"rm4_{li}", name=f"rm4_{li}")
            for bt in range(NBT):
                nc.vector.reduce_max(rm4[:, bt:bt + 1], a_ps[bt][:, :D_H],
                                     axis=AX.X)
            mall = st_pool.tile([128, 1], f32, tag=f"mall{li}")
            nc.vector.reduce_max(mall[:], rm4[:], axis=AX.X)
            nc.vector.tensor_scalar(mall[:], mall[:], 0.0, None, op0=OP.max)
            # scalar AllGather (a [128,1]-strided cc_in DMA pays ~90ns/element
            # descriptor overhead -> 12us; one 4B element is fast)
            pst = ps_sm.tile([1, 128], f32, tag="pmax")
            nc.tensor.transpose(pst[:1, :128], mall[:, :1], identf[:])
            locmax = sc_pool.tile([1, 1], f32, tag=f"locmax{li}",
                                  name=f"locmax{li}")
            nc.vector.reduce_max(locmax[:], pst[:1, :128], axis=AX.X)
            cc_in = dram.tile([1, 1], f32, tag="ccin")
            cc_out = dram.tile([1, NCORES], f32, tag="ccout")
            nc.gpsimd.dma_start(out=cc_in[:], in_=locmax[:])
            nc.gpsimd.collective_compute(
                "AllGather", OP.bypass,
                replica_groups=[list(range(NCORES))],
                ins=[cc_in[:]], outs=[cc_out[:]])

            # --- predicted-Cmax (runs during the gather) ---
            # cmaxp = max(1, PRED2*M_loc^2); c1p = -20/cmaxp; c2p = 10/cmaxp
            # shard-scalar predictor broadcast via PE rank-1 (the per-partition
            # predictor is noisier and needs a second update round -- with the
            # shard predictor one post-gather Newton round suffices)
            c1p3 = sc_pool.tile([1, 3], f32, tag="c1p3", name=f"c1p3_{li}")
            nc.vector.scalar_tensor_tensor(
                c1p3[:, 2:3], locmax[:], PRED2_SHARD, locmax[:],
                op0=OP.mult, op1=OP.mult)
            nc.vector.tensor_scalar(c1p3[:, 2:3], c1p3[:, 2:3], 1.0, None,
                                    op0=OP.max)
            rcp = sc_pool.tile([1, 1], f32, tag="rcp")
            nc.vector.reciprocal(rcp[:], c1p3[:, 2:3])
            nc.vector.tensor_scalar(c1p3[:, 0:1], rcp[:], -20.0, None,
                                    op0=OP.mult)
            nc.vector.tensor_scalar(c1p3[:, 1:2], rcp[:], 10.0, None,
                                    op0=OP.mult)
            ps_bp = ps_sm.tile([128, 3], f32, tag="bcast")
            nc.tensor.matmul(ps_bp[:, :3], ones_col[:1, :128],
                             c1p3[:1, :3], start=True, stop=True)
            cbp = st_pool.tile([128, 3], f32, tag=f"cbp{li}")
            nc.vector.tensor_copy(cbp[:], ps_bp[:, :3])
            c1p_col, c2p_col, cmaxp_col = (cbp[:, 0:1], cbp[:, 1:2],
                                           cbp[:, 2:3])

            # --- packed per-row solver state ---
            B4 = st_pool.tile([128, 4], f32, tag=f"B4_{li}", name=f"B4_{li}")
            for bt in range(NBT):
                nc.vector.tensor_copy(B4[:, bt:bt + 1], c2p_col)
            s04 = st_pool.tile([128, 4], f32, tag=f"s04_{li}", name=f"s04_{li}")
            dneg4 = st_pool.tile([128, 4], f32, tag=f"dn4_{li}")
            dd4 = st_pool.tile([128, 4], f32, tag=f"dd4_{li}")
            rd4 = st_pool.tile([128, 4], f32, tag=f"rd4_{li}")
            u4 = st_pool.tile([128, 4], f32, tag=f"u4_{li}")

            # --- relu copies (fp32 solver input): tiles 0/1 on ACT (Relu is
            # in the same table set as Sigmoid), 2/3 on DVE ---
            a_sb = []
            for bt in range(NBT):
                a = a_pool.tile([128, D_H], f32, tag="a")
                if bt < 3:
                    nc.scalar.activation(a[:], a_ps[bt][:, :D_H], AF.Relu)
                else:
                    nc.vector.tensor_scalar(a[:], a_ps[bt][:, :D_H], 0.0,
                                            None, op0=OP.max)
                a_sb.append(a)

            def deriv_engine(bt):
                # (gpsimd rejects scalar_tensor_tensor at BIR verify; keep DVE)
                return nc.vector

            s0p4 = st_pool.tile([128, 4], f32, tag=f"s0p4_{li}")
            ru4 = st_pool.tile([128, 4], f32, tag=f"ru4_{li}")

            def newton_round(scale_ap, t, per_tile=False, snapshot=False):
                """one Newton round; per_tile=True updates B per batch tile so
                the next ACT pass can start without a barrier.  snapshot saves
                (s0, 1/u) per tile for a following secant round."""
                for bt in range(NBT):
                    y = yb_pool.tile([128, D_H], bf16, tag="yb")
                    nc.scalar.activation(y[:], a_sb[bt][:], AF.Sigmoid,
                                         bias=B4[:, bt:bt + 1], scale=scale_ap,
                                         accum_out=s04[:, bt:bt + 1])
                    t2 = yb_pool.tile([128, D_H], bf16, tag="y2")
                    eng = deriv_engine(bt)
                    eng.scalar_tensor_tensor(
                        t2[:], y[:], 1.0, y[:], op0=OP.subtract, op1=OP.mult,
                        accum_out=dneg4[:, bt:bt + 1])
                    if per_tile:
                        s_ = slice(bt, bt + 1)
                        nc.vector.tensor_scalar(dd4[:, s_], dneg4[:, s_],
                                                -DMIN, None, op0=OP.min)
                        nc.vector.reciprocal(rd4[:, s_], dd4[:, s_])
                        nc.vector.scalar_tensor_tensor(
                            u4[:, s_], s04[:, s_], K_TOPK, rd4[:, s_],
                            op0=OP.subtract, op1=OP.mult)
                        nc.vector.tensor_scalar(u4[:, s_], u4[:, s_], CAP,
                                                -CAP, op0=OP.min, op1=OP.max)
                        nc.vector.tensor_tensor(B4[:, s_], B4[:, s_],
                                                u4[:, s_], op=OP.add)
                        if snapshot:
                            nc.vector.tensor_copy(s0p4[:, s_], s04[:, s_])
                            nc.vector.reciprocal(ru4[:, s_], u4[:, s_])
                if not per_tile:
                    nc.vector.tensor_scalar(dd4[:], dneg4[:], -DMIN, None,
                                            op0=OP.min)
                    nc.vector.reciprocal(rd4[:], dd4[:])
                    nc.vector.scalar_tensor_tensor(u4[:], s04[:], K_TOPK,
                                                   rd4[:], op0=OP.subtract,
                                                   op1=OP.mult)
                    nc.vector.tensor_scalar(u4[:], u4[:], CAP, -CAP,
                                            op0=OP.min, op1=OP.max)
                    nc.vector.tensor_tensor(B4[:], B4[:], u4[:], op=OP.add)
                # dependency-chained dummy matmul: keeps the PE HAM clock
                # warm through the solver so the next burst starts fast
                wp = ps_sm.tile([1, 512], f32, tag="warm")
                nc.tensor.matmul(wp[:1, :64], s04[:, t % 4:t % 4 + 1],
                                 a_sb[t % 4][:, :64], start=True, stop=True)

            def pstate_ramp_chain(n):
                """back-to-back junk streams: the PE p-state only reaches full
                clock after ~3us of CONTINUOUS busy (the L1 burst measured
                383ns/500-col matmul vs 622ns for bursts that start cold).
                Queue-ordered after the solver's last dummy, these keep the PE
                continuously busy through the final rounds so the transpose +
                matmul burst runs at full clock."""
                for _ in range(n):
                    wp = ps_sm.tile([1, 512], f32, tag="warm")
                    nc.tensor.matmul(wp[:1, :512], identb[:, 0:1],
                                     xT3[:, 0, :], start=True, stop=True)

            for t in range(PRE_ROUNDS[li]):
                newton_round(c1p_col, t)
            # prescale half of the warm-start rescale (B *= cmaxp/cmax) while
            # the gather is still in flight; only B *= 1/cmax stays serial
            nc.vector.scalar_tensor_tensor(B4[:], B4[:], cmaxp_col, zero4[:],
                                           op0=OP.mult, op1=OP.add)

            # --- gather result -> global c1 (bcast) ---
            g8 = sc_pool.tile([1, NCORES], f32, tag=f"g8_{li}")
            nc.sync.dma_start(out=g8[:], in_=cc_out[:])
            M = sc_pool.tile([1, 1], f32, tag=f"M{li}")
            nc.vector.reduce_max(M[:], g8[:], axis=AX.X)
            rc = sc_pool.tile([1, 1], f32, tag=f"rc{li}")
            nc.vector.tensor_tensor(rc[:], M[:], M[:], op=OP.mult)
            nc.vector.tensor_scalar(rc[:], rc[:], 1.0, None, op0=OP.max)
            nc.vector.reciprocal(rc[:], rc[:])
            ps_bg = ps_sm.tile([128, 3], f32, tag="bcast")
            nc.tensor.matmul(ps_bg[:, :1], ones_col[:1, :128], rc[:1, :1],
                             start=True, stop=True)
            cbg = st_pool.tile([128, 2], f32, tag=f"cbg{li}")
            nc.vector.tensor_scalar(cbg[:, 0:1], ps_bg[:, 0:1], -20.0, None,
                                    op0=OP.mult)
            nc.vector.scalar_tensor_tensor(B4[:], B4[:], ps_bg[:, 0:1],
                                           zero4[:], op0=OP.mult, op1=OP.add)

            def secant_round(scale_ap):
                """update round without a derivative pass: d from the secant
                of the previous round's (s0, step).  ACT flows straight into
                the finals (per-tile updates, no 500-wide DVE op)."""
                for bt in range(NBT):
                    y = yb_pool.tile([128, D_H], bf16, tag="yb")
                    nc.scalar.activation(y[:], a_sb[bt][:], AF.Sigmoid,
                                         bias=B4[:, bt:bt + 1], scale=scale_ap,
                                         accum_out=s04[:, bt:bt + 1])
                    s_ = slice(bt, bt + 1)
                    nc.vector.tensor_tensor(dd4[:, s_], s0p4[:, s_],
                                            s04[:, s_], op=OP.subtract)
                    nc.vector.tensor_tensor(dd4[:, s_], dd4[:, s_],
                                            ru4[:, s_], op=OP.mult)
                    nc.vector.tensor_scalar(dd4[:, s_], dd4[:, s_], -DMIN,
                                            None, op0=OP.min)
                    nc.vector.reciprocal(rd4[:, s_], dd4[:, s_])
                    nc.vector.scalar_tensor_tensor(
                        u4[:, s_], s04[:, s_], K_TOPK, rd4[:, s_],
                        op0=OP.subtract, op1=OP.mult)
                    nc.vector.tensor_scalar(u4[:, s_], u4[:, s_], CAP, -CAP,
                                            op0=OP.min, op1=OP.max)
                    nc.vector.tensor_tensor(B4[:, s_], B4[:, s_], u4[:, s_],
                                            op=OP.add)

            if UPD_ROUNDS[li] == 1:
                newton_round(cbg[:, 0:1], 1, per_tile=True)
                pstate_ramp_chain(5)
            else:
                newton_round(cbg[:, 0:1], 1, per_tile=True, snapshot=True)
                # p-state ramp: ~10us of continuous PE busy is needed before
                # the transpose+matmul burst runs at full clock; fill the
                # solver tail without running past the start of the burst
                pstate_ramp_chain(5)
                secant_round(cbg[:, 0:1])
                pstate_ramp_chain(6)

            # --- final fp32 eval + mask apply, pipelined per tile ---
            am_tiles = []
            for bt in range(NBT):
                yf = yf_pool.tile([128, D_H], f32, tag="yf")
                nc.scalar.activation(yf[:], a_sb[bt][:], AF.Sigmoid,
                                     bias=B4[:, bt:bt + 1], scale=cbg[:, 0:1],
                                     accum_out=s04[:, bt:bt + 1])
                rs = st_pool.tile([128, 1], f32, tag=f"rs{bt}")
                nc.vector.reciprocal(rs[:], s04[:, bt:bt + 1])
                rsk = st_pool.tile([128, 1], f32, tag=f"rsk{bt}")
                nc.vector.tensor_scalar(rsk[:], rs[:], K_TOPK, None, op0=OP.mult)
                am = am_pool.tile([128, D_H], bf16, tag="am")
                nc.vector.scalar_tensor_tensor(
                    am[:], yf[:], rsk[:, 0:1], a_sb[bt][:],
                    op0=OP.mult, op1=OP.mult)
                am_tiles.append(am)
            return am_tiles

        def transpose_act(am_tiles):
            """[128,500] x4 batch tiles -> amT [125, KC2, 512] bf16"""
            amT = amt_pool.tile([CH, KC2 * BPC], bf16, tag="amT")
            amT3 = amT[:].rearrange("p (c f) -> p c f", c=KC2)
            for bt in range(NBT):
                p = ps_tr.tile([128, KC2 * 128], bf16, tag="tr")
                p3 = p[:].rearrange("p (c f) -> p c f", c=KC2)
                for nck in range(KC2):
                    nc.tensor.transpose(
                        p3[:CH, nck, :],
                        am_tiles[bt][:, nck * CH:(nck + 1) * CH],
                        identb[:])
                dst = amT3[:, :, bt * 128:(bt + 1) * 128]
                nc.scalar.copy(dst, p3[:CH, :, :])
            return amT3

        # ================= the network =================
        def l1_lhs(kk, bt):
            return xT3[:, kk, bt * 128:(bt + 1) * 128]

        a_ps = mm_layer(l1_lhs, W13, brow[0], D_H, KC1)
        am1 = solve_and_mask(a_ps, 1)
        am1T = transpose_act(am1)

        def l2_lhs(kk, bt):
            return am1T[:, kk, bt * 128:(bt + 1) * 128]

        a_ps = mm_layer(l2_lhs, W23, brow[1], D_H, KC2)
        am2 = solve_and_mask(a_ps, 2)
        am2T = transpose_act(am2)

        def l3_lhs(kk, bt):
            return am2T[:, kk, bt * 128:(bt + 1) * 128]

        a_ps = mm_layer(l3_lhs, W33, brow[2], D_H, KC2)
        am3 = solve_and_mask(a_ps, 3)
        am3T = transpose_act(am3)

        # ---- layer 4, transposed: out' [10, 512] = sum_k W4k^T @ am3T_k ----
        po = ps_mm.tile([128, 512], f32, tag="mm")
        for kk in range(KC2):
            nc.tensor.matmul(po[:D_OUT, :BPC], W43[:, kk, :D_OUT],
                             am3T[:, kk, :],
                             start=(kk == 0),
                             stop=(kk == KC2 - 1) and (brow[3] is None))
        if brow[3] is not None:
            nc.tensor.matmul(po[:D_OUT, :BPC], brow[3][:1, :D_OUT],
                             ones_rowb[:1, :BPC], start=False, stop=True)
        out_sb = singles.tile([D_OUT, BPC], f32, tag="osb")
        nc.vector.tensor_copy(out_sb[:], po[:D_OUT, :BPC])
        nc.sync.dma_start(out=out[:], in_=out_sb[:])

    nc.compile()
    return nc


def _get_nc(masked: bool, zero_bias: bool = False):
    key = (masked, zero_bias)
    if key not in _CACHE:
        _CACHE[key] = _build(masked, zero_bias)
    return _CACHE[key]


def build_in_maps(x, W1, b1, W2, b2, W3, b3, W4, b4):
    import ml_dtypes
    bf = ml_dtypes.bfloat16
    x = np.asarray(x, np.float32)
    common = {
        "W1": np.ascontiguousarray(np.asarray(W1, np.float32).astype(bf)),
        "W2": np.ascontiguousarray(np.asarray(W2, np.float32).astype(bf)),
        "W3": np.ascontiguousarray(np.asarray(W3, np.float32).astype(bf)),
        "W4": np.ascontiguousarray(np.asarray(W4, np.float32).astype(bf)),
        "b1": np.asarray(b1, np.float32).reshape(1, D_H).astype(bf),
        "b2": np.asarray(b2, np.float32).reshape(1, D_H).astype(bf),
        "b3": np.asarray(b3, np.float32).reshape(1, D_H).astype(bf),
        "b4": np.asarray(b4, np.float32).reshape(1, D_OUT).astype(bf),
    }
    in_maps = []
    for c in range(NCORES):
        xs = x[c * BPC:(c + 1) * BPC, :]
        in_maps.append(
            {"xT": np.ascontiguousarray(xs.T.astype(bf)), **common})
    return in_maps


def kernel(x, W1, b1, W2, b2, W3, b3, W4, b4, sparse):
    s = float(np.asarray(sparse))
    assert s in (0.0, 1.0), f"sparse must be 0 or 1, got {s}"
    zb = all(not np.any(np.asarray(b)) for b in (b1, b2, b3, b4))
    nc = _get_nc(masked=(s == 1.0), zero_bias=zb)

    in_maps = build_in_maps(x, W1, b1, W2, b2, W3, b3, W4, b4)
    from concourse.bass_utils import run_bass_kernel_spmd
    res = run_bass_kernel_spmd(nc, in_maps, core_ids=list(range(NCORES)))
    return np.concatenate(
        [np.ascontiguousarray(res.results[c]["out"].T) for c in range(NCORES)],
        axis=0)


if __name__ == "__main__":
    rng = np.random.default_rng(0)
    ins = {
        "x": rng.standard_normal((BS, D_IN), np.float32),
        "W1": rng.standard_normal((D_IN, D_H), np.float32) / np.sqrt(D_IN),
        "b1": np.zeros(D_H, np.float32),
        "W2": rng.standard_normal((D_H, D_H), np.float32) / np.sqrt(D_H),
        "b2": np.zeros(D_H, np.float32),
        "W3": rng.standard_normal((D_H, D_H), np.float32) / np.sqrt(D_H),
        "b3": np.zeros(D_H, np.float32),
        "W4": rng.standard_normal((D_H, D_OUT), np.float32) / np.sqrt(D_H),
        "b4": np.zeros(D_OUT, np.float32),
        "sparse": 1,
    }
    o = kernel(**ins)
    print("out", o.shape, o.dtype, np.abs(o).max())
